# revision 23
# baseline (speedup 1.0000x reference)
"""Deformable Conv2D Trainium2 kernel (8-core data-parallel over batch).

Per core (one image, H=W=128, C=64, F=128, 3x3 deformable conv):
  1. offset conv (PE, fp16, K-packed dual-tap matmuls)
  2. offsets transposed to pixel-major (PE identity matmuls)
  3. bilinear weights (interleaved 4-corner layout) + gather indices (DVE)
  4. index fold to the wrapped gather layout (PE transposes) and
     replication to 128 partitions (PE matmul with a 16->128 rep matrix)
  5. ONE dma_gather per 2-row unit of 512B 4-corner tokens from a
     host-prepped duplicated layout z4[y,x] = [x(y,x), x(y+1,x),
     x(y,x+1), x(y+1,x+1)] fp16 -- 1 token per (pixel, tap)
  6. bilinear combine: 1 broadcast-weight multiply + 3 strided adds per
     unit (DVE, fp16) -> sampled fp16
  7. sampled transposed to channel-major via PE identity matmuls into a
     halo'd per-band buffer
  8. main conv: 45 accumulating PE matmuls per 512-px chunk (fp16)
  9. output transposed to pixel-major (PE transpose-mode), DMA'd out

Self-contained: hardcodes shapes for the nn_DeformableConv2D problem.
"""
import os
import numpy as np

import concourse.bass as bass
import concourse.bacc as bacc
import concourse.tile as tile
from concourse import mybir
from concourse.bass_utils import run_bass_kernel_spmd

F32, F16, I16 = mybir.dt.float32, mybir.dt.float16, mybir.dt.int16
ALU = mybir.AluOpType
ACTF = mybir.ActivationFunctionType

H = WD = 128
C = 64
F = 128
T = 9            # deformable taps
NCORES = 8
ROWS_PER_BAND = 8
BANDS = H // ROWS_PER_BAND          # 16
UNITS = 4                            # 2-row units per band
PXROW = WD                            # 128 px per image row
PAD = 130                            # padded row length for shifted reads
KB = 5                               # K blocks of main conv (576 -> 640)
SLOT = PAD                           # 130 cols per row slot in scm
SCMW = KB * 10 * SLOT                # 6500 cols per band buffer
TOK = 256                            # fp16 elems per 4-corner token (512B)
UIDX = 2 * T * PXROW                 # tokens per 2-row unit = 2304

_CACHE = {}


STAGE = int(os.environ.get("KSTAGE", "4"))


def build_program():
    if "nc" in _CACHE:
        return _CACHE["nc"]
    nc = bacc.Bacc("TRN2", target_bir_lowering=False, debug=False)

    # ---- DRAM I/O ----
    z4 = nc.dram_tensor("z4", [H * WD * TOK], F16, kind="ExternalInput").ap()
    xdup = nc.dram_tensor("xdup", [128, PAD * PAD], F16, kind="ExternalInput").ap()
    woffd_in = nc.dram_tensor("woffd", [128, 3 * 18], F16, kind="ExternalInput").ap()
    woffs_in = nc.dram_tensor("woffs", [64, 3 * 18], F16, kind="ExternalInput").ap()
    wm_in = nc.dram_tensor("wm", [128, 45 * 128], F16, kind="ExternalInput").ap()
    cx_in = nc.dram_tensor("cx", [128, H * T], F32, kind="ExternalInput").ap()
    cy_in = nc.dram_tensor("cy", [128, H * T], F32, kind="ExternalInput").ap()
    i32_in = nc.dram_tensor("i128f", [128, 128], F32, kind="ExternalInput").ap()
    i16_in = nc.dram_tensor("i128h", [128, 128], F16, kind="ExternalInput").ap()
    e8_in = nc.dram_tensor("e8", [128, 8 * 128], F32, kind="ExternalInput").ap()
    b_in = nc.dram_tensor("b_main", [128, 1], F32, kind="ExternalInput").ap()
    boff_in = nc.dram_tensor("b_off", [18, 1], F32, kind="ExternalInput").ap()
    out_dram = nc.dram_tensor("out", [H * WD, F], F32, kind="ExternalOutput").ap()
    dbg = nc.dram_tensor("dbg", [128, 4608], F32, kind="ExternalOutput").ap()

    with tile.TileContext(nc) as tc:
        _emit(nc, tc, z4, xdup, woffd_in, woffs_in, wm_in, cx_in, cy_in,
              i32_in, i16_in, e8_in, b_in, boff_in, out_dram, dbg)

    nc.compile()
    _CACHE["nc"] = nc
    return nc


def _emit(nc, tc, z4, xdup_in, woffd_in, woffs_in, wm_in, cx_in, cy_in,
          i32_in, i16_in, e8_in, b_in, boff_in, out_dram, dbg):
    from contextlib import ExitStack
    with ExitStack() as ctx:
        ec = ctx.enter_context
        st = ec(tc.tile_pool(name="static", bufs=1))
        p_offs = ec(tc.tile_pool(name="offs", bufs=3))
        p_offb = ec(tc.tile_pool(name="offb", bufs=2))
        p_math = ec(tc.tile_pool(name="math", bufs=2))
        p_w4 = ec(tc.tile_pool(name="w4c", bufs=2))
        p_idx = ec(tc.tile_pool(name="idx", bufs=2))
        p_gt = ec(tc.tile_pool(name="gt", bufs=4))
        p_P = ec(tc.tile_pool(name="pp", bufs=2))
        p_cmb = ec(tc.tile_pool(name="cmb", bufs=4))
        p_spx = ec(tc.tile_pool(name="spx", bufs=2))
        p_out = ec(tc.tile_pool(name="outp", bufs=2))
        psA = ec(tc.tile_pool(name="psA", bufs=2, space="PSUM"))
        psB = ec(tc.tile_pool(name="psB", bufs=2, space="PSUM"))
        psS = ec(tc.tile_pool(name="psS", bufs=2, space="PSUM"))
        psC = ec(tc.tile_pool(name="psC", bufs=2, space="PSUM"))

        # ---- static loads (HWDGE; keep GpSimd free for gathers) ----
        xdup = st.tile([128, PAD * PAD], F16)
        nc.sync.dma_start(xdup[:], xdup_in)
        woffd = st.tile([128, 54], F16)
        nc.sync.dma_start(woffd[:], woffd_in)
        woffs = st.tile([64, 54], F16)
        nc.sync.dma_start(woffs[:], woffs_in)
        wm = st.tile([128, 45 * 128], F16)
        nc.sync.dma_start(wm[:], wm_in)
        cx = st.tile([128, H * T], F32)
        nc.sync.dma_start(cx[:], cx_in)
        cy = st.tile([128, H * T], F32)
        nc.sync.dma_start(cy[:], cy_in)
        i32 = st.tile([128, 128], F32)
        nc.sync.dma_start(i32[:], i32_in)
        i16t = st.tile([128, 128], F16)
        nc.sync.dma_start(i16t[:], i16_in)
        e8 = st.tile([128, 8 * 128], F32)
        nc.sync.dma_start(e8[:], e8_in)
        bmain = st.tile([128, 1], F32)
        nc.sync.dma_start(bmain[:], b_in)
        boff = st.tile([18, 1], F32)
        nc.sync.dma_start(boff[:], boff_in)

        scm = [st.tile([128, SCMW], F16, tag=f"scm{r}", name=f"scm{r}")
               for r in range(3)]

        tok_src = bass.AP(z4.tensor, 0, [[TOK, H * WD], [1, TOK]])

        def ap_of(tl, off, dims):
            b = tl[:]
            return bass.AP(b.tensor, b.offset + off, [b.ap[0]] + dims)

        def conv_band(b, scm_b):
            """main conv + output transpose for band b reading scm_b."""
            for ch in range(2):          # two 512-px chunks (4 rows each)
                rb = 4 * ch              # starting row within band
                pc = psC.tile([128, 512], F32, tag="conv")
                n_mm = 45
                k = 0
                for s in range(9):
                    sy, sx = s // 3, s % 3
                    for kb in range(KB):
                        kdim = 128 if kb < 4 else 64
                        lhs = wm[0:kdim, (s * KB + kb) * 128:(s * KB + kb + 1) * 128]
                        rhs = ap_of(scm_b, kb * 10 * SLOT + (rb + sy) * SLOT + sx,
                                    [[SLOT, 4], [1, 128]])
                        rhs = bass.AP(rhs.tensor, rhs.offset,
                                      [[rhs.ap[0][0], kdim]] + rhs.ap[1:])
                        nc.tensor.matmul(
                            pc[:].rearrange("f (r x) -> f r x", r=4), lhs, rhs,
                            start=(k == 0), stop=(k == n_mm - 1))
                        k += 1
                outF = p_out.tile([128, 512], F32, tag="outF")
                nc.scalar.activation(outF[:], pc[:], ACTF.Identity,
                                     bias=bmain[:], scale=1.0)
                po = psB.tile([128, 512], F32, tag="b")
                for j in range(4):
                    nc.tensor.transpose(po[:, j * 128:(j + 1) * 128],
                                        outF[:, j * 128:(j + 1) * 128], i32[:])
                osb = p_out.tile([128, 512], F32, tag="osb")
                nc.scalar.activation(osb[:], po[:], ACTF.Copy)
                base = (b * ROWS_PER_BAND + 4 * ch) * PXROW
                dst = bass.AP(out_dram.tensor, base * F,
                              [[F, 128], [PXROW * F, 4], [1, F]])
                nc.sync.dma_start(
                    dst, osb[:].rearrange("p (j f) -> p j f", j=4))

        def front(b):
            """Offsets conv + bilinear weights + gather-index fold for band b.

            Emitted one band ahead of the gather/combine units and boosted in
            scheduler priority so the next band's gather indices are ready
            before the Q7 finishes the current band's gathers.
            Returns (w4cat, idxb) tiles consumed by units(b).
            """
            # ---------- phase A: offsets conv ----------
            offs_cm = []
            for ch in range(2):
                R = b * ROWS_PER_BAND + 4 * ch
                pa = psA.tile([18, 512], F32, tag="a")
                k = 0
                for ky in range(3):
                    rhs_d = ap_of(xdup, (R + ky) * PAD, [[PAD, 4], [1, 128]])
                    nc.tensor.matmul(
                        pa[:].rearrange("m (r x) -> m r x", r=4),
                        woffd[:, ky * 18:(ky + 1) * 18], rhs_d,
                        start=(k == 0), stop=False)
                    k += 1
                    rhs_s = bass.AP(
                        xdup[:].tensor, xdup[:].offset + (R + ky) * PAD + 2,
                        [[xdup[:].ap[0][0], 64], [PAD, 4], [1, 128]])
                    nc.tensor.matmul(
                        pa[:].rearrange("m (r x) -> m r x", r=4),
                        woffs[:, ky * 18:(ky + 1) * 18], rhs_s,
                        start=False, stop=(ky == 2))
                oc = p_offs.tile([18, 512], F32)
                nc.scalar.activation(oc[:], pa[:], ACTF.Identity,
                                     bias=boff[:], scale=1.0)
                offs_cm.append(oc)
            # ---------- offsets transpose to px-major ----------
            pt = psA.tile([128, 144], F32, tag="a")
            for r in range(ROWS_PER_BAND):
                lhs = offs_cm[r // 4][:, (r % 4) * 128:(r % 4 + 1) * 128]
                nc.tensor.matmul(pt[:, r * 18:(r + 1) * 18], lhs, i32[0:18, 0:18],
                                 start=True, stop=True)
            ob = p_offb.tile([128, 144], F32)
            nc.scalar.activation(ob[:], pt[:], ACTF.Copy)

            # ---------- bilinear weights + indices (px-major) ----------
            NW = ROWS_PER_BAND * T  # 72
            offx = ap_of(ob, 0, [[18, 8], [1, 9]])
            offy = ap_of(ob, 9, [[18, 8], [1, 9]])
            cxs = cx[:, b * NW:(b + 1) * NW]
            cys = cy[:, b * NW:(b + 1) * NW]

            def floor_block(off_ap, cs, hi_clip):
                # All single-ALU fp32 ops, no int round-trip: i16->f32 copies
                # and dual MAX,MIN crawl ~10-50x when a Q7 gather is active
                # (SBUF port contention), plain adds/muls/compares do not.
                l = p_math.tile([128, NW], F32, tag="l")
                nc.vector.tensor_tensor(l[:], off_ap, cs, ALU.add)
                nc.vector.tensor_scalar(l[:], l[:], 0.0, None, ALU.max)
                nc.vector.tensor_scalar(l[:], l[:], float(hi_clip), None,
                                        ALU.min)
                # floor(l) = round_ne(l - 0.5) via the 2^23 magic number;
                # lands in [0, hi-1] so no upper-clip of x0 is needed, and at
                # integral l the (x0=l-1, fx=1) split is bilinear-equivalent
                x0 = p_math.tile([128, NW], F32, tag="x0")
                nc.vector.tensor_scalar(x0[:], l[:], -0.5, None, ALU.add)
                nc.vector.tensor_scalar(x0[:], x0[:], 12582912.0, None, ALU.add)
                nc.vector.tensor_scalar(x0[:], x0[:], -12582912.0, None, ALU.add)
                fx = p_math.tile([128, NW], F32, tag="fx")
                nc.vector.tensor_tensor(fx[:], l[:], x0[:], ALU.subtract)
                # reference zeroes BOTH x-corners when l == hi exactly
                mlt = p_math.tile([128, NW], F32, tag="mx")
                nc.vector.tensor_scalar(mlt[:], l[:], float(hi_clip), None,
                                        ALU.is_lt)
                fxm = p_math.tile([128, NW], F16, tag="fxm")
                nc.vector.tensor_tensor(fxm[:], fx[:], mlt[:], ALU.mult)
                wxa = p_math.tile([128, NW], F16, tag="wxa")
                nc.vector.tensor_scalar(wxa[:], fx[:], -1.0, 1.0, ALU.mult,
                                        ALU.add)
                return x0, fxm, wxa

            x0, fx, wxa = floor_block(offx, cxs, 127)
            y0, fy, wya = floor_block(offy, cys, 127)

            # 4-corner weights interleaved: w4cat[px, 36*r + 4*t + corner]
            # corner order matches z4 token: [y0x0, y1x0, y0x1, y1x1]
            w4cat = p_w4.tile([128, 4 * NW], F16, tag="w4cat")

            def wdst(corner):
                return ap_of(w4cat, corner, [[36, 8], [4, 9]])

            nc.vector.tensor_tensor(wdst(0), wxa[:], wya[:], ALU.mult)
            nc.vector.tensor_tensor(wdst(1), wxa[:], fy[:], ALU.mult)
            nc.vector.tensor_tensor(wdst(2), fx[:], wya[:], ALU.mult)
            nc.vector.tensor_tensor(wdst(3), fx[:], fy[:], ALU.mult)

            i0f = p_w4.tile([128, NW], F32, tag="i0f")
            nc.vector.scalar_tensor_tensor(i0f[:], y0[:], 128.0, x0[:],
                                           op0=ALU.mult, op1=ALU.add)

            # ---------- index fold to wrapped gather layout ----------
            # token i of the band: i = (r*T + t)*128 + px;
            # idx lives at partition px%16, col i//16 = 72*r + 8*t + px//16.
            # One permutation matmul per px-group a: out_a[p, (r,t)] =
            #   i0f[16a + p%16, (r,t)], then strided casts into idxb.
            idxb = p_idx.tile([128, 576], I16)
            for hh in range(2):
                pr = psA.tile([128, 288], F32, tag="a")
                for aa in range(4):
                    a = 4 * hh + aa
                    nc.tensor.matmul(pr[:, aa * 72:(aa + 1) * 72],
                                     e8[:, a * 128:(a + 1) * 128], i0f[:],
                                     start=True, stop=True)
                # dst col = 72r + 8t + a, src col = 72*aa + 9r + t
                dst = ap_of(idxb, 4 * hh, [[72, 8], [8, 9], [1, 4]])
                src = bass.AP(pr[:].tensor, pr[:].offset,
                              [pr[:].ap[0], [9, 8], [1, 9], [72, 4]])
                nc.vector.tensor_copy(dst, src)
            if STAGE <= 1 and b == 0:
                dbgw = p_w4.tile([128, 288], F32, tag="dbgw")
                nc.vector.tensor_copy(dbgw[:], w4cat[:])
                nc.sync.dma_start(dbg[:, 0:288], dbgw[:])
                nc.sync.dma_start(dbg[:, 288:360], i0f[:])
                dbg16 = p_w4.tile([128, 576], F32, tag="dbg16")
                nc.vector.tensor_copy(dbg16[:], idxb[:])
                nc.sync.dma_start(dbg[:, 576:1152], dbg16[:])
            return w4cat, idxb

        def front_hi(b):
            with tc.high_priority(offset=400):
                return front(b)

        fr = front_hi(0)
        for r in range(3):
            nc.vector.memset(scm[r][:], 0)
        for b in range(BANDS):
            scm_b = scm[b % 3]
            w4cat, idxb = fr
            if b + 1 < BANDS:
                fr = front_hi(b + 1)
            if STAGE <= 1:
                continue

            # ---------- per 2-row unit: gather + combine + sampledT ----------
            for u in range(UNITS):
                gt = p_gt.tile([128, 18 * TOK], F16)
                # >1024 idxs per instr needs single_packet=False (HW cap else)
                nc.gpsimd.dma_gather(
                    out_ap=gt[:].rearrange("p (g e) -> p g e", g=18),
                    in_ap=tok_src,
                    idxs_ap=idxb[:, u * 144:(u + 1) * 144],
                    num_idxs=UIDX,
                    num_idxs_reg=UIDX,
                    elem_size=TOK,
                    elem_step=TOK,
                    single_packet=False,
                )
                if STAGE <= 2:
                    if b == 0 and u == 0:
                        dbg16g = p_P.tile([128, 4608], F32, tag="dbgg")
                        nc.vector.tensor_copy(dbg16g[:], gt[:])
                        nc.sync.dma_start(dbg[:, 0:4608], dbg16g[:])
                    continue
                # combine per row-half: gt *= w4 (broadcast 64, in place),
                # then sum the 4 corners
                spx = p_spx.tile([128, 2 * 576], F16)
                for hi in range(2):
                    wsrc = ap_of(w4cat, 72 * u + 36 * hi, [[1, 36], [0, 64]])
                    gh = bass.AP(gt[:].tensor, gt[:].offset + 2304 * hi,
                                 [gt[:].ap[0], [1, 36 * 64]])
                    nc.vector.tensor_tensor(
                        gh.rearrange("p (tc c) -> p tc c", c=64), gh.rearrange(
                            "p (tc c) -> p tc c", c=64), wsrc, ALU.mult)

                    def cs(corner):
                        return ap_of(gt, 2304 * hi + corner * 64,
                                     [[256, 9], [1, 64]])

                    t_ab = p_cmb.tile([128, 576], F16, tag="tab")
                    nc.vector.tensor_tensor(
                        t_ab[:].rearrange("p (t c) -> p t c", c=64),
                        cs(0), cs(1), ALU.add)
                    t_cd = p_cmb.tile([128, 576], F16, tag="tcd")
                    nc.vector.tensor_tensor(
                        t_cd[:].rearrange("p (t c) -> p t c", c=64),
                        cs(2), cs(3), ALU.add)
                    nc.vector.tensor_tensor(spx[:, hi * 576:(hi + 1) * 576],
                                            t_ab[:], t_cd[:], ALU.add)

                # sampled transpose to channel-major; boundary rows are also
                # written into the neighbor band's halo slot (replaces
                # explicit halo copies)
                for hi in range(2):
                    slot = 2 * u + hi + 1
                    ps1 = psS.tile([128, 512], F32, tag="s1")
                    ps2 = psB.tile([64, 128], F32, tag="b")
                    for kb in range(4):
                        nc.tensor.matmul(
                            ps1[:, kb * 128:(kb + 1) * 128],
                            spx[:, hi * 576 + kb * 128: hi * 576 + (kb + 1) * 128],
                            i16t[:], start=True, stop=True)
                    nc.tensor.matmul(ps2[:], spx[:, hi * 576 + 512:hi * 576 + 576],
                                     i16t[:], start=True, stop=True)
                    targets = [(scm_b, slot)]
                    if u == 0 and hi == 0 and b > 0:
                        targets.append((scm[(b - 1) % 3], 9))
                    if u == UNITS - 1 and hi == 1 and b + 1 < BANDS:
                        targets.append((scm[(b + 1) % 3], 0))
                    for (scm_t, sl) in targets:
                        dst1 = ap_of(scm_t, sl * SLOT + 1,
                                     [[10 * SLOT, 4], [1, 128]])
                        nc.scalar.activation(dst1, ps1[:].rearrange(
                            "p (k x) -> p k x", k=4), ACTF.Copy)
                        dst2 = bass.AP(scm_t[:].tensor,
                                       scm_t[:].offset + 4 * 10 * SLOT
                                       + sl * SLOT + 1,
                                       [[scm_t[:].ap[0][0], 64], [1, 128]])
                        nc.scalar.activation(dst2, ps2[:], ACTF.Copy)

            if STAGE <= 2:
                continue
            if STAGE <= 3:
                if b == 0:
                    sdbg = p_out.tile([128, 4608], F32, tag="sdbg")
                    nc.vector.tensor_copy(sdbg[:], scm_b[:, 0:4608])
                    nc.sync.dma_start(dbg[:], sdbg[:])
                continue
            # halo slots are filled by the boundary-row double-writes above
            if b > 0:
                conv_band(b - 1, scm[(b - 1) % 3])
            if b == BANDS - 1:
                nc.vector.memset(
                    ap_of(scm_b, 9 * SLOT, [[10 * SLOT, KB], [1, SLOT]]), 0)
                conv_band(b, scm_b)


def _host_prep(x_img, W_off, b_off, W, b):
    """Build per-core input map. x_img: (128,128,64) fp32."""
    C_, T_ = C, T
    # 4-corner duplicated token layout: z4[y, x] =
    #   [x(y,x,:), x(y+1,x,:), x(y,x+1,:), x(y+1,x+1,:)]  (zeros past edges)
    xh = np.ascontiguousarray(x_img, np.float32).astype(np.float16)
    z4 = np.zeros((H, WD, 4, C_), np.float16)
    z4[:, :, 0] = xh
    z4[:H - 1, :, 1] = xh[1:]
    z4[:, :WD - 1, 2] = xh[:, 1:]
    z4[:H - 1, :WD - 1, 3] = xh[1:, 1:]
    z4 = z4.reshape(H * WD * TOK)

    # padded transposed image + dup(+1 col) for offset conv
    xT = np.zeros((C_, PAD, PAD), np.float16)
    xT[:, 1:129, 1:129] = np.transpose(x_img, (2, 0, 1)).astype(np.float16)
    xT = xT.reshape(C_, PAD * PAD)
    xdup = np.zeros((128, PAD * PAD), np.float16)
    xdup[:C_] = xT
    xdup[C_:, :PAD * PAD - 1] = xT[:, 1:]

    perm = list(range(0, 18, 2)) + list(range(1, 18, 2))
    woffd = np.zeros((128, 3 * 18), np.float16)
    woffs = np.zeros((64, 3 * 18), np.float16)
    for ky in range(3):
        woffd[:C_, ky * 18:(ky + 1) * 18] = W_off[ky, 0][:, perm].astype(np.float16)
        woffd[C_:, ky * 18:(ky + 1) * 18] = W_off[ky, 1][:, perm].astype(np.float16)
        woffs[:, ky * 18:(ky + 1) * 18] = W_off[ky, 2][:, perm].astype(np.float16)

    wm = np.zeros((128, 45 * 128), np.float16)
    for s in range(9):
        blk = W[s // 3, s % 3].astype(np.float16)        # [576, 128]
        for kb in range(KB):
            kd = 128 if kb < 4 else 64
            wm[:kd, (s * KB + kb) * 128:(s * KB + kb + 1) * 128] = \
                blk[kb * 128: kb * 128 + kd]

    lo = np.arange(128, dtype=np.float32)
    hi = np.arange(H, dtype=np.float32)
    t = np.arange(T_)
    kx = (t % 3 - 1).astype(np.float32)
    ky = (t // 3 - 1).astype(np.float32)
    cx = (lo[:, None, None] + kx[None, None, :] +
          np.zeros((1, H, 1), np.float32)).reshape(128, H * T_)
    cy = (np.zeros((128, 1, 1), np.float32) + hi[None, :, None] +
          ky[None, None, :]).reshape(128, H * T_)

    e8 = np.zeros((128, 8 * 128), np.float32)
    pp = np.arange(128)
    for a in range(8):
        e8[16 * a + pp % 16, 128 * a + pp] = 1.0

    return dict(
        z4=z4,
        xdup=xdup,
        woffd=woffd,
        woffs=woffs,
        wm=wm,
        cx=np.ascontiguousarray(cx),
        cy=np.ascontiguousarray(cy),
        i128f=np.eye(128, dtype=np.float32),
        i128h=np.eye(128, dtype=np.float16),
        e8=e8,
        b_main=np.asarray(b, np.float32).reshape(128, 1),
        b_off=np.asarray(b_off, np.float32)[
            list(range(0, 18, 2)) + list(range(1, 18, 2))].reshape(18, 1),
    )


def kernel(x, W_off, b_off, W, b, _trace=False):
    x = np.asarray(x, np.float32)
    nc = build_program()
    in_maps = [_host_prep(x[i], np.asarray(W_off, np.float32),
                          np.asarray(b_off, np.float32),
                          np.asarray(W, np.float32),
                          np.asarray(b, np.float32))
               for i in range(NCORES)]
    res = run_bass_kernel_spmd(nc, in_maps, list(range(NCORES)), trace=_trace)
    out = np.stack([res.results[i]["out"].reshape(H, WD, F)
                    for i in range(NCORES)])
    if _trace:
        kernel.last_exec_time_ns = res.exec_time_ns
        kernel.last_results = res
    return out


kernel.last_exec_time_ns = None


# revision 25
# speedup vs baseline: 1.0043x; 1.0043x over previous
"""Deformable Conv2D Trainium2 kernel (8-core data-parallel over batch).

Per core (one image, H=W=128, C=64, F=128, 3x3 deformable conv):
  1. offset conv (PE, fp16, K-packed dual-tap matmuls)
  2. offsets transposed to pixel-major (PE identity matmuls)
  3. bilinear weights (interleaved 4-corner layout) + gather indices (DVE)
  4. index fold to the wrapped gather layout (PE transposes) and
     replication to 128 partitions (PE matmul with a 16->128 rep matrix)
  5. ONE dma_gather per 2-row unit of 512B 4-corner tokens from a
     host-prepped duplicated layout z4[y,x] = [x(y,x), x(y+1,x),
     x(y,x+1), x(y+1,x+1)] fp16 -- 1 token per (pixel, tap)
  6. bilinear combine: 1 broadcast-weight multiply + 3 strided adds per
     unit (DVE, fp16) -> sampled fp16
  7. sampled transposed to channel-major via PE identity matmuls into a
     halo'd per-band buffer
  8. main conv: 45 accumulating PE matmuls per 512-px chunk (fp16)
  9. output transposed to pixel-major (PE transpose-mode), DMA'd out

Self-contained: hardcodes shapes for the nn_DeformableConv2D problem.
"""
import os
import numpy as np

import concourse.bass as bass
import concourse.bacc as bacc
import concourse.tile as tile
from concourse import mybir
from concourse.bass_utils import run_bass_kernel_spmd

F32, F16, I16 = mybir.dt.float32, mybir.dt.float16, mybir.dt.int16
ALU = mybir.AluOpType
ACTF = mybir.ActivationFunctionType

H = WD = 128
C = 64
F = 128
T = 9            # deformable taps
NCORES = 8
ROWS_PER_BAND = 8
BANDS = H // ROWS_PER_BAND          # 16
UNITS = 4                            # 2-row units per band
PXROW = WD                            # 128 px per image row
PAD = 130                            # padded row length for shifted reads
KB = 5                               # K blocks of main conv (576 -> 640)
SLOT = PAD                           # 130 cols per row slot in scm
SCMW = KB * 10 * SLOT                # 6500 cols per band buffer
TOK = 256                            # fp16 elems per 4-corner token (512B)
UIDX = 2 * T * PXROW                 # tokens per 2-row unit = 2304

_CACHE = {}


STAGE = int(os.environ.get("KSTAGE", "4"))


def build_program():
    if "nc" in _CACHE:
        return _CACHE["nc"]
    nc = bacc.Bacc("TRN2", target_bir_lowering=False, debug=False)

    # ---- DRAM I/O ----
    z4 = nc.dram_tensor("z4", [H * WD * TOK], F16, kind="ExternalInput").ap()
    xdup = nc.dram_tensor("xdup", [128, PAD * PAD], F16, kind="ExternalInput").ap()
    woffd_in = nc.dram_tensor("woffd", [128, 3 * 18], F16, kind="ExternalInput").ap()
    woffs_in = nc.dram_tensor("woffs", [64, 3 * 18], F16, kind="ExternalInput").ap()
    wm_in = nc.dram_tensor("wm", [128, 45 * 128], F16, kind="ExternalInput").ap()
    cx_in = nc.dram_tensor("cx", [128, H * T], F32, kind="ExternalInput").ap()
    cy_in = nc.dram_tensor("cy", [128, H * T], F32, kind="ExternalInput").ap()
    i32_in = nc.dram_tensor("i128f", [128, 128], F32, kind="ExternalInput").ap()
    i16_in = nc.dram_tensor("i128h", [128, 128], F16, kind="ExternalInput").ap()
    e8_in = nc.dram_tensor("e8", [128, 8 * 128], F32, kind="ExternalInput").ap()
    b_in = nc.dram_tensor("b_main", [128, 1], F32, kind="ExternalInput").ap()
    boff_in = nc.dram_tensor("b_off", [18, 1], F32, kind="ExternalInput").ap()
    out_dram = nc.dram_tensor("out", [H * WD, F], F32, kind="ExternalOutput").ap()
    dbg = nc.dram_tensor("dbg", [128, 4608], F32, kind="ExternalOutput").ap()

    with tile.TileContext(nc) as tc:
        _emit(nc, tc, z4, xdup, woffd_in, woffs_in, wm_in, cx_in, cy_in,
              i32_in, i16_in, e8_in, b_in, boff_in, out_dram, dbg)

    nc.compile()
    _CACHE["nc"] = nc
    return nc


def _emit(nc, tc, z4, xdup_in, woffd_in, woffs_in, wm_in, cx_in, cy_in,
          i32_in, i16_in, e8_in, b_in, boff_in, out_dram, dbg):
    from contextlib import ExitStack
    with ExitStack() as ctx:
        ec = ctx.enter_context
        st = ec(tc.tile_pool(name="static", bufs=1))
        p_offs = ec(tc.tile_pool(name="offs", bufs=3))
        p_offb = ec(tc.tile_pool(name="offb", bufs=2))
        p_math = ec(tc.tile_pool(name="math", bufs=2))
        p_w4 = ec(tc.tile_pool(name="w4c", bufs=2))
        p_idx = ec(tc.tile_pool(name="idx", bufs=2))
        p_gt = ec(tc.tile_pool(name="gt", bufs=4))
        p_P = ec(tc.tile_pool(name="pp", bufs=2))
        p_cmb = ec(tc.tile_pool(name="cmb", bufs=4))
        p_spx = ec(tc.tile_pool(name="spx", bufs=2))
        p_out = ec(tc.tile_pool(name="outp", bufs=2))
        psA = ec(tc.tile_pool(name="psA", bufs=2, space="PSUM"))
        psB = ec(tc.tile_pool(name="psB", bufs=2, space="PSUM"))
        psS = ec(tc.tile_pool(name="psS", bufs=2, space="PSUM"))
        psC = ec(tc.tile_pool(name="psC", bufs=2, space="PSUM"))

        # ---- static loads (HWDGE; keep GpSimd free for gathers) ----
        xdup = st.tile([128, PAD * PAD], F16)
        nc.sync.dma_start(xdup[:], xdup_in)
        woffd = st.tile([128, 54], F16)
        nc.sync.dma_start(woffd[:], woffd_in)
        woffs = st.tile([64, 54], F16)
        nc.sync.dma_start(woffs[:], woffs_in)
        wm = st.tile([128, 45 * 128], F16)
        nc.sync.dma_start(wm[:], wm_in)
        cx = st.tile([128, H * T], F32)
        nc.sync.dma_start(cx[:], cx_in)
        cy = st.tile([128, H * T], F32)
        nc.sync.dma_start(cy[:], cy_in)
        i32 = st.tile([128, 128], F32)
        nc.sync.dma_start(i32[:], i32_in)
        i16t = st.tile([128, 128], F16)
        nc.sync.dma_start(i16t[:], i16_in)
        e8 = st.tile([128, 8 * 128], F32)
        nc.sync.dma_start(e8[:], e8_in)
        bmain = st.tile([128, 1], F32)
        nc.sync.dma_start(bmain[:], b_in)
        boff = st.tile([18, 1], F32)
        nc.sync.dma_start(boff[:], boff_in)

        scm = [st.tile([128, SCMW], F16, tag=f"scm{r}", name=f"scm{r}")
               for r in range(3)]

        tok_src = bass.AP(z4.tensor, 0, [[TOK, H * WD], [1, TOK]])

        def ap_of(tl, off, dims):
            b = tl[:]
            return bass.AP(b.tensor, b.offset + off, [b.ap[0]] + dims)

        def conv_band(b, scm_b):
            """main conv + output transpose for band b reading scm_b."""
            for ch in range(2):          # two 512-px chunks (4 rows each)
                rb = 4 * ch              # starting row within band
                pc = psC.tile([128, 512], F32, tag="conv")
                n_mm = 45
                k = 0
                for s in range(9):
                    sy, sx = s // 3, s % 3
                    for kb in range(KB):
                        kdim = 128 if kb < 4 else 64
                        lhs = wm[0:kdim, (s * KB + kb) * 128:(s * KB + kb + 1) * 128]
                        rhs = ap_of(scm_b, kb * 10 * SLOT + (rb + sy) * SLOT + sx,
                                    [[SLOT, 4], [1, 128]])
                        rhs = bass.AP(rhs.tensor, rhs.offset,
                                      [[rhs.ap[0][0], kdim]] + rhs.ap[1:])
                        nc.tensor.matmul(
                            pc[:].rearrange("f (r x) -> f r x", r=4), lhs, rhs,
                            start=(k == 0), stop=(k == n_mm - 1))
                        k += 1
                outF = p_out.tile([128, 512], F32, tag="outF")
                nc.scalar.activation(outF[:], pc[:], ACTF.Identity,
                                     bias=bmain[:], scale=1.0)
                po = psB.tile([128, 512], F32, tag="b")
                for j in range(4):
                    nc.tensor.transpose(po[:, j * 128:(j + 1) * 128],
                                        outF[:, j * 128:(j + 1) * 128], i32[:])
                osb = p_out.tile([128, 512], F32, tag="osb")
                nc.scalar.activation(osb[:], po[:], ACTF.Copy)
                base = (b * ROWS_PER_BAND + 4 * ch) * PXROW
                dst = bass.AP(out_dram.tensor, base * F,
                              [[F, 128], [PXROW * F, 4], [1, F]])
                nc.sync.dma_start(
                    dst, osb[:].rearrange("p (j f) -> p j f", j=4))

        def front(b):
            """Offsets conv + bilinear weights + gather-index fold for band b.

            Emitted one band ahead of the gather/combine units and boosted in
            scheduler priority so the next band's gather indices are ready
            before the Q7 finishes the current band's gathers.
            Returns (w4cat, idxb) tiles consumed by units(b).
            """
            # ---------- phase A: offsets conv ----------
            offs_cm = []
            for ch in range(2):
                R = b * ROWS_PER_BAND + 4 * ch
                pa = psA.tile([18, 512], F32, tag="a")
                k = 0
                for ky in range(3):
                    rhs_d = ap_of(xdup, (R + ky) * PAD, [[PAD, 4], [1, 128]])
                    nc.tensor.matmul(
                        pa[:].rearrange("m (r x) -> m r x", r=4),
                        woffd[:, ky * 18:(ky + 1) * 18], rhs_d,
                        start=(k == 0), stop=False)
                    k += 1
                    rhs_s = bass.AP(
                        xdup[:].tensor, xdup[:].offset + (R + ky) * PAD + 2,
                        [[xdup[:].ap[0][0], 64], [PAD, 4], [1, 128]])
                    nc.tensor.matmul(
                        pa[:].rearrange("m (r x) -> m r x", r=4),
                        woffs[:, ky * 18:(ky + 1) * 18], rhs_s,
                        start=False, stop=(ky == 2))
                oc = p_offs.tile([18, 512], F32)
                nc.scalar.activation(oc[:], pa[:], ACTF.Identity,
                                     bias=boff[:], scale=1.0)
                offs_cm.append(oc)
            # ---------- offsets transpose to px-major ----------
            pt = psA.tile([128, 144], F32, tag="a")
            for r in range(ROWS_PER_BAND):
                lhs = offs_cm[r // 4][:, (r % 4) * 128:(r % 4 + 1) * 128]
                nc.tensor.matmul(pt[:, r * 18:(r + 1) * 18], lhs, i32[0:18, 0:18],
                                 start=True, stop=True)
            ob = p_offb.tile([128, 144], F32)
            nc.scalar.activation(ob[:], pt[:], ACTF.Copy)

            # ---------- bilinear weights + indices (px-major) ----------
            NW = ROWS_PER_BAND * T  # 72
            offx = ap_of(ob, 0, [[18, 8], [1, 9]])
            offy = ap_of(ob, 9, [[18, 8], [1, 9]])
            cxs = cx[:, b * NW:(b + 1) * NW]
            cys = cy[:, b * NW:(b + 1) * NW]

            def floor_block(off_ap, cs, hi_clip):
                # All single-ALU fp32 ops, no int round-trip: i16->f32 copies
                # and dual MAX,MIN crawl ~10-50x when a Q7 gather is active
                # (SBUF port contention), plain adds/muls/compares do not.
                # clamp(l, 0, hi) without MAX/MIN ops (those crawl too):
                # low via l*(l>0), high via mask-blend, mask mh reused below
                l = p_math.tile([128, NW], F32, tag="l")
                nc.vector.tensor_tensor(l[:], off_ap, cs, ALU.add)
                lw = p_math.tile([128, NW], F32, tag="lw")
                nc.vector.tensor_scalar(lw[:], l[:], 0.0, None, ALU.is_gt)
                nc.vector.tensor_tensor(l[:], l[:], lw[:], ALU.mult)
                mh = p_math.tile([128, NW], F32, tag="mh")
                nc.vector.tensor_scalar(mh[:], l[:], float(hi_clip), None,
                                        ALU.is_lt)
                nc.vector.tensor_tensor(l[:], l[:], mh[:], ALU.mult)
                t2 = p_math.tile([128, NW], F32, tag="t2")
                nc.vector.tensor_scalar(t2[:], mh[:], float(-hi_clip),
                                        float(hi_clip), ALU.mult, ALU.add)
                nc.vector.tensor_tensor(l[:], l[:], t2[:], ALU.add)
                # floor(l) = round_ne(l - 0.5) via the 2^23 magic number;
                # lands in [0, hi-1] so no upper-clip of x0 is needed, and at
                # integral l the (x0=l-1, fx=1) split is bilinear-equivalent
                x0 = p_math.tile([128, NW], F32, tag="x0")
                nc.vector.tensor_scalar(x0[:], l[:], -0.5, None, ALU.add)
                nc.vector.tensor_scalar(x0[:], x0[:], 12582912.0, None, ALU.add)
                nc.vector.tensor_scalar(x0[:], x0[:], -12582912.0, None, ALU.add)
                fx = p_math.tile([128, NW], F32, tag="fx")
                nc.vector.tensor_tensor(fx[:], l[:], x0[:], ALU.subtract)
                # reference zeroes BOTH x-corners when l == hi exactly;
                # mh == (l < hi) is exactly that mask
                fxm = p_math.tile([128, NW], F16, tag="fxm")
                nc.vector.tensor_tensor(fxm[:], fx[:], mh[:], ALU.mult)
                wxa = p_math.tile([128, NW], F16, tag="wxa")
                nc.vector.tensor_scalar(wxa[:], fx[:], -1.0, 1.0, ALU.mult,
                                        ALU.add)
                return x0, fxm, wxa

            x0, fx, wxa = floor_block(offx, cxs, 127)
            y0, fy, wya = floor_block(offy, cys, 127)

            # 4-corner weights interleaved: w4cat[px, 36*r + 4*t + corner]
            # corner order matches z4 token: [y0x0, y1x0, y0x1, y1x1]
            w4cat = p_w4.tile([128, 4 * NW], F16, tag="w4cat")

            def wdst(corner):
                return ap_of(w4cat, corner, [[36, 8], [4, 9]])

            nc.vector.tensor_tensor(wdst(0), wxa[:], wya[:], ALU.mult)
            nc.vector.tensor_tensor(wdst(1), wxa[:], fy[:], ALU.mult)
            nc.vector.tensor_tensor(wdst(2), fx[:], wya[:], ALU.mult)
            nc.vector.tensor_tensor(wdst(3), fx[:], fy[:], ALU.mult)

            i0f = p_w4.tile([128, NW], F32, tag="i0f")
            nc.vector.scalar_tensor_tensor(i0f[:], y0[:], 128.0, x0[:],
                                           op0=ALU.mult, op1=ALU.add)

            # ---------- index fold to wrapped gather layout ----------
            # token i of the band: i = (r*T + t)*128 + px;
            # idx lives at partition px%16, col i//16 = 72*r + 8*t + px//16.
            # One permutation matmul per px-group a: out_a[p, (r,t)] =
            #   i0f[16a + p%16, (r,t)], then strided casts into idxb.
            idxb = p_idx.tile([128, 576], I16)
            for hh in range(2):
                pr = psA.tile([128, 288], F32, tag="a")
                for aa in range(4):
                    a = 4 * hh + aa
                    nc.tensor.matmul(pr[:, aa * 72:(aa + 1) * 72],
                                     e8[:, a * 128:(a + 1) * 128], i0f[:],
                                     start=True, stop=True)
                # dst col = 72r + 8t + a, src col = 72*aa + 9r + t
                dst = ap_of(idxb, 4 * hh, [[72, 8], [8, 9], [1, 4]])
                src = bass.AP(pr[:].tensor, pr[:].offset,
                              [pr[:].ap[0], [9, 8], [1, 9], [72, 4]])
                nc.vector.tensor_copy(dst, src)
            if STAGE <= 1 and b == 0:
                dbgw = p_w4.tile([128, 288], F32, tag="dbgw")
                nc.vector.tensor_copy(dbgw[:], w4cat[:])
                nc.sync.dma_start(dbg[:, 0:288], dbgw[:])
                nc.sync.dma_start(dbg[:, 288:360], i0f[:])
                dbg16 = p_w4.tile([128, 576], F32, tag="dbg16")
                nc.vector.tensor_copy(dbg16[:], idxb[:])
                nc.sync.dma_start(dbg[:, 576:1152], dbg16[:])
            return w4cat, idxb

        def front_hi(b):
            with tc.high_priority(offset=400):
                return front(b)

        fr = front_hi(0)
        for r in range(3):
            nc.vector.memset(scm[r][:], 0)
        for b in range(BANDS):
            scm_b = scm[b % 3]
            w4cat, idxb = fr
            if b + 1 < BANDS:
                fr = front_hi(b + 1)
            if STAGE <= 1:
                continue

            # ---------- per 2-row unit: gather + combine + sampledT ----------
            for u in range(UNITS):
                gt = p_gt.tile([128, 18 * TOK], F16)
                # >1024 idxs per instr needs single_packet=False (HW cap else)
                nc.gpsimd.dma_gather(
                    out_ap=gt[:].rearrange("p (g e) -> p g e", g=18),
                    in_ap=tok_src,
                    idxs_ap=idxb[:, u * 144:(u + 1) * 144],
                    num_idxs=UIDX,
                    num_idxs_reg=UIDX,
                    elem_size=TOK,
                    elem_step=TOK,
                    single_packet=False,
                )
                if STAGE <= 2:
                    if b == 0 and u == 0:
                        dbg16g = p_P.tile([128, 4608], F32, tag="dbgg")
                        nc.vector.tensor_copy(dbg16g[:], gt[:])
                        nc.sync.dma_start(dbg[:, 0:4608], dbg16g[:])
                    continue
                # combine per row-half: gt *= w4 (broadcast 64, in place),
                # then sum the 4 corners
                spx = p_spx.tile([128, 2 * 576], F16)
                for hi in range(2):
                    wsrc = ap_of(w4cat, 72 * u + 36 * hi, [[1, 36], [0, 64]])
                    gh = bass.AP(gt[:].tensor, gt[:].offset + 2304 * hi,
                                 [gt[:].ap[0], [1, 36 * 64]])
                    nc.vector.tensor_tensor(
                        gh.rearrange("p (tc c) -> p tc c", c=64), gh.rearrange(
                            "p (tc c) -> p tc c", c=64), wsrc, ALU.mult)

                    def cs(corner):
                        return ap_of(gt, 2304 * hi + corner * 64,
                                     [[256, 9], [1, 64]])

                    t_ab = p_cmb.tile([128, 576], F16, tag="tab")
                    nc.vector.tensor_tensor(
                        t_ab[:].rearrange("p (t c) -> p t c", c=64),
                        cs(0), cs(1), ALU.add)
                    t_cd = p_cmb.tile([128, 576], F16, tag="tcd")
                    nc.vector.tensor_tensor(
                        t_cd[:].rearrange("p (t c) -> p t c", c=64),
                        cs(2), cs(3), ALU.add)
                    nc.vector.tensor_tensor(spx[:, hi * 576:(hi + 1) * 576],
                                            t_ab[:], t_cd[:], ALU.add)

                # sampled transpose to channel-major; boundary rows are also
                # written into the neighbor band's halo slot (replaces
                # explicit halo copies)
                for hi in range(2):
                    slot = 2 * u + hi + 1
                    ps1 = psS.tile([128, 512], F32, tag="s1")
                    ps2 = psB.tile([64, 128], F32, tag="b")
                    for kb in range(4):
                        nc.tensor.matmul(
                            ps1[:, kb * 128:(kb + 1) * 128],
                            spx[:, hi * 576 + kb * 128: hi * 576 + (kb + 1) * 128],
                            i16t[:], start=True, stop=True)
                    nc.tensor.matmul(ps2[:], spx[:, hi * 576 + 512:hi * 576 + 576],
                                     i16t[:], start=True, stop=True)
                    targets = [(scm_b, slot)]
                    if u == 0 and hi == 0 and b > 0:
                        targets.append((scm[(b - 1) % 3], 9))
                    if u == UNITS - 1 and hi == 1 and b + 1 < BANDS:
                        targets.append((scm[(b + 1) % 3], 0))
                    for (scm_t, sl) in targets:
                        dst1 = ap_of(scm_t, sl * SLOT + 1,
                                     [[10 * SLOT, 4], [1, 128]])
                        nc.scalar.activation(dst1, ps1[:].rearrange(
                            "p (k x) -> p k x", k=4), ACTF.Copy)
                        dst2 = bass.AP(scm_t[:].tensor,
                                       scm_t[:].offset + 4 * 10 * SLOT
                                       + sl * SLOT + 1,
                                       [[scm_t[:].ap[0][0], 64], [1, 128]])
                        nc.scalar.activation(dst2, ps2[:], ACTF.Copy)

            if STAGE <= 2:
                continue
            if STAGE <= 3:
                if b == 0:
                    sdbg = p_out.tile([128, 4608], F32, tag="sdbg")
                    nc.vector.tensor_copy(sdbg[:], scm_b[:, 0:4608])
                    nc.sync.dma_start(dbg[:], sdbg[:])
                continue
            # halo slots are filled by the boundary-row double-writes above
            if b > 0:
                conv_band(b - 1, scm[(b - 1) % 3])
            if b == BANDS - 1:
                nc.vector.memset(
                    ap_of(scm_b, 9 * SLOT, [[10 * SLOT, KB], [1, SLOT]]), 0)
                conv_band(b, scm_b)


def _host_prep(x_img, W_off, b_off, W, b):
    """Build per-core input map. x_img: (128,128,64) fp32."""
    C_, T_ = C, T
    # 4-corner duplicated token layout: z4[y, x] =
    #   [x(y,x,:), x(y+1,x,:), x(y,x+1,:), x(y+1,x+1,:)]  (zeros past edges)
    xh = np.ascontiguousarray(x_img, np.float32).astype(np.float16)
    z4 = np.zeros((H, WD, 4, C_), np.float16)
    z4[:, :, 0] = xh
    z4[:H - 1, :, 1] = xh[1:]
    z4[:, :WD - 1, 2] = xh[:, 1:]
    z4[:H - 1, :WD - 1, 3] = xh[1:, 1:]
    z4 = z4.reshape(H * WD * TOK)

    # padded transposed image + dup(+1 col) for offset conv
    xT = np.zeros((C_, PAD, PAD), np.float16)
    xT[:, 1:129, 1:129] = np.transpose(x_img, (2, 0, 1)).astype(np.float16)
    xT = xT.reshape(C_, PAD * PAD)
    xdup = np.zeros((128, PAD * PAD), np.float16)
    xdup[:C_] = xT
    xdup[C_:, :PAD * PAD - 1] = xT[:, 1:]

    perm = list(range(0, 18, 2)) + list(range(1, 18, 2))
    woffd = np.zeros((128, 3 * 18), np.float16)
    woffs = np.zeros((64, 3 * 18), np.float16)
    for ky in range(3):
        woffd[:C_, ky * 18:(ky + 1) * 18] = W_off[ky, 0][:, perm].astype(np.float16)
        woffd[C_:, ky * 18:(ky + 1) * 18] = W_off[ky, 1][:, perm].astype(np.float16)
        woffs[:, ky * 18:(ky + 1) * 18] = W_off[ky, 2][:, perm].astype(np.float16)

    wm = np.zeros((128, 45 * 128), np.float16)
    for s in range(9):
        blk = W[s // 3, s % 3].astype(np.float16)        # [576, 128]
        for kb in range(KB):
            kd = 128 if kb < 4 else 64
            wm[:kd, (s * KB + kb) * 128:(s * KB + kb + 1) * 128] = \
                blk[kb * 128: kb * 128 + kd]

    lo = np.arange(128, dtype=np.float32)
    hi = np.arange(H, dtype=np.float32)
    t = np.arange(T_)
    kx = (t % 3 - 1).astype(np.float32)
    ky = (t // 3 - 1).astype(np.float32)
    cx = (lo[:, None, None] + kx[None, None, :] +
          np.zeros((1, H, 1), np.float32)).reshape(128, H * T_)
    cy = (np.zeros((128, 1, 1), np.float32) + hi[None, :, None] +
          ky[None, None, :]).reshape(128, H * T_)

    e8 = np.zeros((128, 8 * 128), np.float32)
    pp = np.arange(128)
    for a in range(8):
        e8[16 * a + pp % 16, 128 * a + pp] = 1.0

    return dict(
        z4=z4,
        xdup=xdup,
        woffd=woffd,
        woffs=woffs,
        wm=wm,
        cx=np.ascontiguousarray(cx),
        cy=np.ascontiguousarray(cy),
        i128f=np.eye(128, dtype=np.float32),
        i128h=np.eye(128, dtype=np.float16),
        e8=e8,
        b_main=np.asarray(b, np.float32).reshape(128, 1),
        b_off=np.asarray(b_off, np.float32)[
            list(range(0, 18, 2)) + list(range(1, 18, 2))].reshape(18, 1),
    )


def kernel(x, W_off, b_off, W, b, _trace=False):
    x = np.asarray(x, np.float32)
    nc = build_program()
    in_maps = [_host_prep(x[i], np.asarray(W_off, np.float32),
                          np.asarray(b_off, np.float32),
                          np.asarray(W, np.float32),
                          np.asarray(b, np.float32))
               for i in range(NCORES)]
    res = run_bass_kernel_spmd(nc, in_maps, list(range(NCORES)), trace=_trace)
    out = np.stack([res.results[i]["out"].reshape(H, WD, F)
                    for i in range(NCORES)])
    if _trace:
        kernel.last_exec_time_ns = res.exec_time_ns
        kernel.last_results = res
    return out


kernel.last_exec_time_ns = None


# revision 27
# speedup vs baseline: 1.0126x; 1.0083x over previous
"""Deformable Conv2D Trainium2 kernel (8-core data-parallel over batch).

Per core (one image, H=W=128, C=64, F=128, 3x3 deformable conv):
  1. offset conv (PE, fp16, K-packed dual-tap matmuls)
  2. offsets transposed to pixel-major (PE identity matmuls)
  3. bilinear weights (interleaved 4-corner layout) + gather indices (DVE)
  4. index fold to the wrapped gather layout (PE transposes) and
     replication to 128 partitions (PE matmul with a 16->128 rep matrix)
  5. ONE dma_gather per 2-row unit of 512B 4-corner tokens from a
     host-prepped duplicated layout z4[y,x] = [x(y,x), x(y+1,x),
     x(y,x+1), x(y+1,x+1)] fp16 -- 1 token per (pixel, tap)
  6. bilinear combine: 1 broadcast-weight multiply + 3 strided adds per
     unit (DVE, fp16) -> sampled fp16
  7. sampled transposed to channel-major via PE identity matmuls into a
     halo'd per-band buffer
  8. main conv: 45 accumulating PE matmuls per 512-px chunk (fp16)
  9. output transposed to pixel-major (PE transpose-mode), DMA'd out

Self-contained: hardcodes shapes for the nn_DeformableConv2D problem.
"""
import os
import numpy as np

import concourse.bass as bass
import concourse.bacc as bacc
import concourse.tile as tile
from concourse import mybir
from concourse.bass_utils import run_bass_kernel_spmd

F32, F16, I16 = mybir.dt.float32, mybir.dt.float16, mybir.dt.int16
ALU = mybir.AluOpType
ACTF = mybir.ActivationFunctionType

H = WD = 128
C = 64
F = 128
T = 9            # deformable taps
NCORES = 8
ROWS_PER_BAND = 8
BANDS = H // ROWS_PER_BAND          # 16
UNITS = 4                            # 2-row units per band
PXROW = WD                            # 128 px per image row
PAD = 130                            # padded row length for shifted reads
KB = 5                               # K blocks of main conv (576 -> 640)
SLOT = PAD                           # 130 cols per row slot in scm
SCMW = KB * 10 * SLOT                # 6500 cols per band buffer
TOK = 256                            # fp16 elems per 4-corner token (512B)
UIDX = 2 * T * PXROW                 # tokens per 2-row unit = 2304

_CACHE = {}


STAGE = int(os.environ.get("KSTAGE", "4"))


def build_program():
    if "nc" in _CACHE:
        return _CACHE["nc"]
    # The scheduler's cost model prices SWDGE gather descriptor generation at
    # 0.34 ns/desc; measured on HW it is ~7.9 ns/desc. With the default the
    # scheduler thinks gathers are ~20x cheaper than reality and orders
    # gather-waits ahead of independent work, stalling the pipeline. Patch
    # for the duration of trace/compile (schedule hint only; restored after).
    from concourse import hw_specs
    _spec = hw_specs.get_hw_spec("TRN2")
    _orig_ns = _spec.SWDGE_NS_PER_DESCRIPTOR
    _spec.SWDGE_NS_PER_DESCRIPTOR = 7.93
    try:
        nc = _build_program_inner()
    finally:
        _spec.SWDGE_NS_PER_DESCRIPTOR = _orig_ns
    _CACHE["nc"] = nc
    return nc


def _build_program_inner():
    nc = bacc.Bacc("TRN2", target_bir_lowering=False, debug=False)

    # ---- DRAM I/O ----
    z4 = nc.dram_tensor("z4", [H * WD * TOK], F16, kind="ExternalInput").ap()
    xdup = nc.dram_tensor("xdup", [128, PAD * PAD], F16, kind="ExternalInput").ap()
    woffd_in = nc.dram_tensor("woffd", [128, 3 * 18], F16, kind="ExternalInput").ap()
    woffs_in = nc.dram_tensor("woffs", [64, 3 * 18], F16, kind="ExternalInput").ap()
    wm_in = nc.dram_tensor("wm", [128, 45 * 128], F16, kind="ExternalInput").ap()
    cx_in = nc.dram_tensor("cx", [128, H * T], F32, kind="ExternalInput").ap()
    cy_in = nc.dram_tensor("cy", [128, H * T], F32, kind="ExternalInput").ap()
    i32_in = nc.dram_tensor("i128f", [128, 128], F32, kind="ExternalInput").ap()
    i16_in = nc.dram_tensor("i128h", [128, 128], F16, kind="ExternalInput").ap()
    e8_in = nc.dram_tensor("e8", [128, 8 * 128], F32, kind="ExternalInput").ap()
    b_in = nc.dram_tensor("b_main", [128, 1], F32, kind="ExternalInput").ap()
    boff_in = nc.dram_tensor("b_off", [18, 1], F32, kind="ExternalInput").ap()
    out_dram = nc.dram_tensor("out", [H * WD, F], F32, kind="ExternalOutput").ap()
    dbg = nc.dram_tensor("dbg", [128, 4608], F32, kind="ExternalOutput").ap()

    with tile.TileContext(nc) as tc:
        _emit(nc, tc, z4, xdup, woffd_in, woffs_in, wm_in, cx_in, cy_in,
              i32_in, i16_in, e8_in, b_in, boff_in, out_dram, dbg)

    nc.compile()
    return nc


def _emit(nc, tc, z4, xdup_in, woffd_in, woffs_in, wm_in, cx_in, cy_in,
          i32_in, i16_in, e8_in, b_in, boff_in, out_dram, dbg):
    from contextlib import ExitStack
    with ExitStack() as ctx:
        ec = ctx.enter_context
        st = ec(tc.tile_pool(name="static", bufs=1))
        p_offs = ec(tc.tile_pool(name="offs", bufs=3))
        p_offb = ec(tc.tile_pool(name="offb", bufs=2))
        p_math = ec(tc.tile_pool(name="math", bufs=2))
        p_w4 = ec(tc.tile_pool(name="w4c", bufs=2))
        p_idx = ec(tc.tile_pool(name="idx", bufs=2))
        p_gt = ec(tc.tile_pool(name="gt", bufs=4))
        p_P = ec(tc.tile_pool(name="pp", bufs=2))
        p_cmb = ec(tc.tile_pool(name="cmb", bufs=4))
        p_spx = ec(tc.tile_pool(name="spx", bufs=2))
        p_out = ec(tc.tile_pool(name="outp", bufs=2))
        psA = ec(tc.tile_pool(name="psA", bufs=2, space="PSUM"))
        psB = ec(tc.tile_pool(name="psB", bufs=2, space="PSUM"))
        psS = ec(tc.tile_pool(name="psS", bufs=2, space="PSUM"))
        psC = ec(tc.tile_pool(name="psC", bufs=2, space="PSUM"))

        # ---- static loads (HWDGE; keep GpSimd free for gathers) ----
        xdup = st.tile([128, PAD * PAD], F16)
        nc.sync.dma_start(xdup[:], xdup_in)
        woffd = st.tile([128, 54], F16)
        nc.sync.dma_start(woffd[:], woffd_in)
        woffs = st.tile([64, 54], F16)
        nc.sync.dma_start(woffs[:], woffs_in)
        wm = st.tile([128, 45 * 128], F16)
        nc.sync.dma_start(wm[:], wm_in)
        cx = st.tile([128, H * T], F32)
        nc.sync.dma_start(cx[:], cx_in)
        cy = st.tile([128, H * T], F32)
        nc.sync.dma_start(cy[:], cy_in)
        i32 = st.tile([128, 128], F32)
        nc.sync.dma_start(i32[:], i32_in)
        i16t = st.tile([128, 128], F16)
        nc.sync.dma_start(i16t[:], i16_in)
        e8 = st.tile([128, 8 * 128], F32)
        nc.sync.dma_start(e8[:], e8_in)
        bmain = st.tile([128, 1], F32)
        nc.sync.dma_start(bmain[:], b_in)
        boff = st.tile([18, 1], F32)
        nc.sync.dma_start(boff[:], boff_in)

        scm = [st.tile([128, SCMW], F16, tag=f"scm{r}", name=f"scm{r}")
               for r in range(3)]

        tok_src = bass.AP(z4.tensor, 0, [[TOK, H * WD], [1, TOK]])

        def ap_of(tl, off, dims):
            b = tl[:]
            return bass.AP(b.tensor, b.offset + off, [b.ap[0]] + dims)

        def conv_band(b, scm_b):
            """main conv + output transpose for band b reading scm_b."""
            for ch in range(2):          # two 512-px chunks (4 rows each)
                rb = 4 * ch              # starting row within band
                pc = psC.tile([128, 512], F32, tag="conv")
                n_mm = 45
                k = 0
                for s in range(9):
                    sy, sx = s // 3, s % 3
                    for kb in range(KB):
                        kdim = 128 if kb < 4 else 64
                        lhs = wm[0:kdim, (s * KB + kb) * 128:(s * KB + kb + 1) * 128]
                        rhs = ap_of(scm_b, kb * 10 * SLOT + (rb + sy) * SLOT + sx,
                                    [[SLOT, 4], [1, 128]])
                        rhs = bass.AP(rhs.tensor, rhs.offset,
                                      [[rhs.ap[0][0], kdim]] + rhs.ap[1:])
                        nc.tensor.matmul(
                            pc[:].rearrange("f (r x) -> f r x", r=4), lhs, rhs,
                            start=(k == 0), stop=(k == n_mm - 1))
                        k += 1
                outF = p_out.tile([128, 512], F32, tag="outF")
                nc.scalar.activation(outF[:], pc[:], ACTF.Identity,
                                     bias=bmain[:], scale=1.0)
                po = psB.tile([128, 512], F32, tag="b")
                for j in range(4):
                    nc.tensor.transpose(po[:, j * 128:(j + 1) * 128],
                                        outF[:, j * 128:(j + 1) * 128], i32[:])
                osb = p_out.tile([128, 512], F32, tag="osb")
                nc.scalar.activation(osb[:], po[:], ACTF.Copy)
                base = (b * ROWS_PER_BAND + 4 * ch) * PXROW
                dst = bass.AP(out_dram.tensor, base * F,
                              [[F, 128], [PXROW * F, 4], [1, F]])
                nc.sync.dma_start(
                    dst, osb[:].rearrange("p (j f) -> p j f", j=4))

        def front(b):
            """Offsets conv + bilinear weights + gather-index fold for band b.

            Emitted one band ahead of the gather/combine units and boosted in
            scheduler priority so the next band's gather indices are ready
            before the Q7 finishes the current band's gathers.
            Returns (w4cat, idxb) tiles consumed by units(b).
            """
            # ---------- phase A: offsets conv ----------
            offs_cm = []
            for ch in range(2):
                R = b * ROWS_PER_BAND + 4 * ch
                pa = psA.tile([18, 512], F32, tag="a")
                k = 0
                for ky in range(3):
                    rhs_d = ap_of(xdup, (R + ky) * PAD, [[PAD, 4], [1, 128]])
                    nc.tensor.matmul(
                        pa[:].rearrange("m (r x) -> m r x", r=4),
                        woffd[:, ky * 18:(ky + 1) * 18], rhs_d,
                        start=(k == 0), stop=False)
                    k += 1
                    rhs_s = bass.AP(
                        xdup[:].tensor, xdup[:].offset + (R + ky) * PAD + 2,
                        [[xdup[:].ap[0][0], 64], [PAD, 4], [1, 128]])
                    nc.tensor.matmul(
                        pa[:].rearrange("m (r x) -> m r x", r=4),
                        woffs[:, ky * 18:(ky + 1) * 18], rhs_s,
                        start=False, stop=(ky == 2))
                oc = p_offs.tile([18, 512], F32)
                nc.scalar.activation(oc[:], pa[:], ACTF.Identity,
                                     bias=boff[:], scale=1.0)
                offs_cm.append(oc)
            # ---------- offsets transpose to px-major ----------
            pt = psA.tile([128, 144], F32, tag="a")
            for r in range(ROWS_PER_BAND):
                lhs = offs_cm[r // 4][:, (r % 4) * 128:(r % 4 + 1) * 128]
                nc.tensor.matmul(pt[:, r * 18:(r + 1) * 18], lhs, i32[0:18, 0:18],
                                 start=True, stop=True)
            ob = p_offb.tile([128, 144], F32)
            nc.scalar.activation(ob[:], pt[:], ACTF.Copy)

            # ---------- bilinear weights + indices (px-major) ----------
            NW = ROWS_PER_BAND * T  # 72
            offx = ap_of(ob, 0, [[18, 8], [1, 9]])
            offy = ap_of(ob, 9, [[18, 8], [1, 9]])
            cxs = cx[:, b * NW:(b + 1) * NW]
            cys = cy[:, b * NW:(b + 1) * NW]

            def floor_block(off_ap, cs, hi_clip):
                # All single-ALU fp32 ops, no int round-trip: i16->f32 copies
                # and dual MAX,MIN crawl ~10-50x when a Q7 gather is active
                # (SBUF port contention), plain adds/muls/compares do not.
                # clamp(l, 0, hi) without MAX/MIN ops (those crawl too):
                # low via l*(l>0), high via mask-blend, mask mh reused below
                l = p_math.tile([128, NW], F32, tag="l")
                nc.vector.tensor_tensor(l[:], off_ap, cs, ALU.add)
                lw = p_math.tile([128, NW], F32, tag="lw")
                nc.vector.tensor_scalar(lw[:], l[:], 0.0, None, ALU.is_gt)
                nc.vector.tensor_tensor(l[:], l[:], lw[:], ALU.mult)
                mh = p_math.tile([128, NW], F32, tag="mh")
                nc.vector.tensor_scalar(mh[:], l[:], float(hi_clip), None,
                                        ALU.is_lt)
                nc.vector.tensor_tensor(l[:], l[:], mh[:], ALU.mult)
                t2 = p_math.tile([128, NW], F32, tag="t2")
                nc.vector.tensor_scalar(t2[:], mh[:], float(-hi_clip),
                                        float(hi_clip), ALU.mult, ALU.add)
                nc.vector.tensor_tensor(l[:], l[:], t2[:], ALU.add)
                # floor(l) = round_ne(l - 0.5) via the 2^23 magic number;
                # lands in [0, hi-1] so no upper-clip of x0 is needed, and at
                # integral l the (x0=l-1, fx=1) split is bilinear-equivalent
                x0 = p_math.tile([128, NW], F32, tag="x0")
                nc.vector.tensor_scalar(x0[:], l[:], -0.5, None, ALU.add)
                nc.vector.tensor_scalar(x0[:], x0[:], 12582912.0, None, ALU.add)
                nc.vector.tensor_scalar(x0[:], x0[:], -12582912.0, None, ALU.add)
                fx = p_math.tile([128, NW], F32, tag="fx")
                nc.vector.tensor_tensor(fx[:], l[:], x0[:], ALU.subtract)
                # reference zeroes BOTH x-corners when l == hi exactly;
                # mh == (l < hi) is exactly that mask
                fxm = p_math.tile([128, NW], F16, tag="fxm")
                nc.vector.tensor_tensor(fxm[:], fx[:], mh[:], ALU.mult)
                wxa = p_math.tile([128, NW], F16, tag="wxa")
                nc.vector.tensor_scalar(wxa[:], fx[:], -1.0, 1.0, ALU.mult,
                                        ALU.add)
                return x0, fxm, wxa

            x0, fx, wxa = floor_block(offx, cxs, 127)
            y0, fy, wya = floor_block(offy, cys, 127)

            # 4-corner weights interleaved: w4cat[px, 36*r + 4*t + corner]
            # corner order matches z4 token: [y0x0, y1x0, y0x1, y1x1]
            w4cat = p_w4.tile([128, 4 * NW], F16, tag="w4cat")

            def wdst(corner):
                return ap_of(w4cat, corner, [[36, 8], [4, 9]])

            nc.vector.tensor_tensor(wdst(0), wxa[:], wya[:], ALU.mult)
            nc.vector.tensor_tensor(wdst(1), wxa[:], fy[:], ALU.mult)
            nc.vector.tensor_tensor(wdst(2), fx[:], wya[:], ALU.mult)
            nc.vector.tensor_tensor(wdst(3), fx[:], fy[:], ALU.mult)

            i0f = p_w4.tile([128, NW], F32, tag="i0f")
            nc.vector.scalar_tensor_tensor(i0f[:], y0[:], 128.0, x0[:],
                                           op0=ALU.mult, op1=ALU.add)

            # ---------- index fold to wrapped gather layout ----------
            # token i of the band: i = (r*T + t)*128 + px;
            # idx lives at partition px%16, col i//16 = 72*r + 8*t + px//16.
            # One permutation matmul per px-group a: out_a[p, (r,t)] =
            #   i0f[16a + p%16, (r,t)], then strided casts into idxb.
            idxb = p_idx.tile([128, 576], I16)
            for hh in range(2):
                pr = psA.tile([128, 288], F32, tag="a")
                for aa in range(4):
                    a = 4 * hh + aa
                    nc.tensor.matmul(pr[:, aa * 72:(aa + 1) * 72],
                                     e8[:, a * 128:(a + 1) * 128], i0f[:],
                                     start=True, stop=True)
                # dst col = 72r + 8t + a, src col = 72*aa + 9r + t
                dst = ap_of(idxb, 4 * hh, [[72, 8], [8, 9], [1, 4]])
                src = bass.AP(pr[:].tensor, pr[:].offset,
                              [pr[:].ap[0], [9, 8], [1, 9], [72, 4]])
                nc.vector.tensor_copy(dst, src)
            if STAGE <= 1 and b == 0:
                dbgw = p_w4.tile([128, 288], F32, tag="dbgw")
                nc.vector.tensor_copy(dbgw[:], w4cat[:])
                nc.sync.dma_start(dbg[:, 0:288], dbgw[:])
                nc.sync.dma_start(dbg[:, 288:360], i0f[:])
                dbg16 = p_w4.tile([128, 576], F32, tag="dbg16")
                nc.vector.tensor_copy(dbg16[:], idxb[:])
                nc.sync.dma_start(dbg[:, 576:1152], dbg16[:])
            return w4cat, idxb

        def front_hi(b):
            with tc.high_priority(offset=400):
                return front(b)

        fr = front_hi(0)
        for r in range(3):
            nc.vector.memset(scm[r][:], 0)
        for b in range(BANDS):
            scm_b = scm[b % 3]
            w4cat, idxb = fr
            if b + 1 < BANDS:
                fr = front_hi(b + 1)
            if STAGE <= 1:
                continue

            # ---------- per 2-row unit: gather + combine + sampledT ----------
            for u in range(UNITS):
                gt = p_gt.tile([128, 18 * TOK], F16)
                # >1024 idxs per instr needs single_packet=False (HW cap else)
                nc.gpsimd.dma_gather(
                    out_ap=gt[:].rearrange("p (g e) -> p g e", g=18),
                    in_ap=tok_src,
                    idxs_ap=idxb[:, u * 144:(u + 1) * 144],
                    num_idxs=UIDX,
                    num_idxs_reg=UIDX,
                    elem_size=TOK,
                    elem_step=TOK,
                    single_packet=False,
                )
                if STAGE <= 2:
                    if b == 0 and u == 0:
                        dbg16g = p_P.tile([128, 4608], F32, tag="dbgg")
                        nc.vector.tensor_copy(dbg16g[:], gt[:])
                        nc.sync.dma_start(dbg[:, 0:4608], dbg16g[:])
                    continue
                # combine per row-half: gt *= w4 (broadcast 64, in place),
                # then sum the 4 corners
                spx = p_spx.tile([128, 2 * 576], F16)
                for hi in range(2):
                    wsrc = ap_of(w4cat, 72 * u + 36 * hi, [[1, 36], [0, 64]])
                    gh = bass.AP(gt[:].tensor, gt[:].offset + 2304 * hi,
                                 [gt[:].ap[0], [1, 36 * 64]])
                    nc.vector.tensor_tensor(
                        gh.rearrange("p (tc c) -> p tc c", c=64), gh.rearrange(
                            "p (tc c) -> p tc c", c=64), wsrc, ALU.mult)

                    def cs(corner):
                        return ap_of(gt, 2304 * hi + corner * 64,
                                     [[256, 9], [1, 64]])

                    t_ab = p_cmb.tile([128, 576], F16, tag="tab")
                    nc.vector.tensor_tensor(
                        t_ab[:].rearrange("p (t c) -> p t c", c=64),
                        cs(0), cs(1), ALU.add)
                    t_cd = p_cmb.tile([128, 576], F16, tag="tcd")
                    nc.vector.tensor_tensor(
                        t_cd[:].rearrange("p (t c) -> p t c", c=64),
                        cs(2), cs(3), ALU.add)
                    nc.vector.tensor_tensor(spx[:, hi * 576:(hi + 1) * 576],
                                            t_ab[:], t_cd[:], ALU.add)

                # sampled transpose to channel-major; boundary rows are also
                # written into the neighbor band's halo slot (replaces
                # explicit halo copies)
                for hi in range(2):
                    slot = 2 * u + hi + 1
                    ps1 = psS.tile([128, 512], F32, tag="s1")
                    ps2 = psB.tile([64, 128], F32, tag="b")
                    for kb in range(4):
                        nc.tensor.matmul(
                            ps1[:, kb * 128:(kb + 1) * 128],
                            spx[:, hi * 576 + kb * 128: hi * 576 + (kb + 1) * 128],
                            i16t[:], start=True, stop=True)
                    nc.tensor.matmul(ps2[:], spx[:, hi * 576 + 512:hi * 576 + 576],
                                     i16t[:], start=True, stop=True)
                    targets = [(scm_b, slot)]
                    if u == 0 and hi == 0 and b > 0:
                        targets.append((scm[(b - 1) % 3], 9))
                    if u == UNITS - 1 and hi == 1 and b + 1 < BANDS:
                        targets.append((scm[(b + 1) % 3], 0))
                    for (scm_t, sl) in targets:
                        dst1 = ap_of(scm_t, sl * SLOT + 1,
                                     [[10 * SLOT, 4], [1, 128]])
                        nc.scalar.activation(dst1, ps1[:].rearrange(
                            "p (k x) -> p k x", k=4), ACTF.Copy)
                        dst2 = bass.AP(scm_t[:].tensor,
                                       scm_t[:].offset + 4 * 10 * SLOT
                                       + sl * SLOT + 1,
                                       [[scm_t[:].ap[0][0], 64], [1, 128]])
                        nc.scalar.activation(dst2, ps2[:], ACTF.Copy)

            if STAGE <= 2:
                continue
            if STAGE <= 3:
                if b == 0:
                    sdbg = p_out.tile([128, 4608], F32, tag="sdbg")
                    nc.vector.tensor_copy(sdbg[:], scm_b[:, 0:4608])
                    nc.sync.dma_start(dbg[:], sdbg[:])
                continue
            # halo slots are filled by the boundary-row double-writes above
            if b > 0:
                conv_band(b - 1, scm[(b - 1) % 3])
            if b == BANDS - 1:
                nc.vector.memset(
                    ap_of(scm_b, 9 * SLOT, [[10 * SLOT, KB], [1, SLOT]]), 0)
                conv_band(b, scm_b)


def _host_prep(x_img, W_off, b_off, W, b):
    """Build per-core input map. x_img: (128,128,64) fp32."""
    C_, T_ = C, T
    # 4-corner duplicated token layout: z4[y, x] =
    #   [x(y,x,:), x(y+1,x,:), x(y,x+1,:), x(y+1,x+1,:)]  (zeros past edges)
    xh = np.ascontiguousarray(x_img, np.float32).astype(np.float16)
    z4 = np.zeros((H, WD, 4, C_), np.float16)
    z4[:, :, 0] = xh
    z4[:H - 1, :, 1] = xh[1:]
    z4[:, :WD - 1, 2] = xh[:, 1:]
    z4[:H - 1, :WD - 1, 3] = xh[1:, 1:]
    z4 = z4.reshape(H * WD * TOK)

    # padded transposed image + dup(+1 col) for offset conv
    xT = np.zeros((C_, PAD, PAD), np.float16)
    xT[:, 1:129, 1:129] = np.transpose(x_img, (2, 0, 1)).astype(np.float16)
    xT = xT.reshape(C_, PAD * PAD)
    xdup = np.zeros((128, PAD * PAD), np.float16)
    xdup[:C_] = xT
    xdup[C_:, :PAD * PAD - 1] = xT[:, 1:]

    perm = list(range(0, 18, 2)) + list(range(1, 18, 2))
    woffd = np.zeros((128, 3 * 18), np.float16)
    woffs = np.zeros((64, 3 * 18), np.float16)
    for ky in range(3):
        woffd[:C_, ky * 18:(ky + 1) * 18] = W_off[ky, 0][:, perm].astype(np.float16)
        woffd[C_:, ky * 18:(ky + 1) * 18] = W_off[ky, 1][:, perm].astype(np.float16)
        woffs[:, ky * 18:(ky + 1) * 18] = W_off[ky, 2][:, perm].astype(np.float16)

    wm = np.zeros((128, 45 * 128), np.float16)
    for s in range(9):
        blk = W[s // 3, s % 3].astype(np.float16)        # [576, 128]
        for kb in range(KB):
            kd = 128 if kb < 4 else 64
            wm[:kd, (s * KB + kb) * 128:(s * KB + kb + 1) * 128] = \
                blk[kb * 128: kb * 128 + kd]

    lo = np.arange(128, dtype=np.float32)
    hi = np.arange(H, dtype=np.float32)
    t = np.arange(T_)
    kx = (t % 3 - 1).astype(np.float32)
    ky = (t // 3 - 1).astype(np.float32)
    cx = (lo[:, None, None] + kx[None, None, :] +
          np.zeros((1, H, 1), np.float32)).reshape(128, H * T_)
    cy = (np.zeros((128, 1, 1), np.float32) + hi[None, :, None] +
          ky[None, None, :]).reshape(128, H * T_)

    e8 = np.zeros((128, 8 * 128), np.float32)
    pp = np.arange(128)
    for a in range(8):
        e8[16 * a + pp % 16, 128 * a + pp] = 1.0

    return dict(
        z4=z4,
        xdup=xdup,
        woffd=woffd,
        woffs=woffs,
        wm=wm,
        cx=np.ascontiguousarray(cx),
        cy=np.ascontiguousarray(cy),
        i128f=np.eye(128, dtype=np.float32),
        i128h=np.eye(128, dtype=np.float16),
        e8=e8,
        b_main=np.asarray(b, np.float32).reshape(128, 1),
        b_off=np.asarray(b_off, np.float32)[
            list(range(0, 18, 2)) + list(range(1, 18, 2))].reshape(18, 1),
    )


def kernel(x, W_off, b_off, W, b, _trace=False):
    x = np.asarray(x, np.float32)
    nc = build_program()
    in_maps = [_host_prep(x[i], np.asarray(W_off, np.float32),
                          np.asarray(b_off, np.float32),
                          np.asarray(W, np.float32),
                          np.asarray(b, np.float32))
               for i in range(NCORES)]
    res = run_bass_kernel_spmd(nc, in_maps, list(range(NCORES)), trace=_trace)
    out = np.stack([res.results[i]["out"].reshape(H, WD, F)
                    for i in range(NCORES)])
    if _trace:
        kernel.last_exec_time_ns = res.exec_time_ns
        kernel.last_results = res
    return out


kernel.last_exec_time_ns = None


# revision 29
# speedup vs baseline: 1.0311x; 1.0183x over previous
"""Deformable Conv2D Trainium2 kernel (8-core data-parallel over batch).

Per core (one image, H=W=128, C=64, F=128, 3x3 deformable conv):
  1. offset conv (PE, fp16, K-packed dual-tap matmuls)
  2. offsets transposed to pixel-major (PE identity matmuls)
  3. bilinear weights (interleaved 4-corner layout) + gather indices (DVE)
  4. index fold to the wrapped gather layout (PE transposes) and
     replication to 128 partitions (PE matmul with a 16->128 rep matrix)
  5. ONE dma_gather per 2-row unit of 512B 4-corner tokens from a
     host-prepped duplicated layout z4[y,x] = [x(y,x), x(y+1,x),
     x(y,x+1), x(y+1,x+1)] fp16 -- 1 token per (pixel, tap)
  6. bilinear combine: 1 broadcast-weight multiply + 3 strided adds per
     unit (DVE, fp16) -> sampled fp16
  7. sampled transposed to channel-major via PE identity matmuls into a
     halo'd per-band buffer
  8. main conv: 45 accumulating PE matmuls per 512-px chunk (fp16)
  9. output transposed to pixel-major (PE transpose-mode), DMA'd out

Self-contained: hardcodes shapes for the nn_DeformableConv2D problem.
"""
import os
import numpy as np

import concourse.bass as bass
import concourse.bacc as bacc
import concourse.tile as tile
from concourse import mybir
from concourse.bass_utils import run_bass_kernel_spmd

F32, F16, I16 = mybir.dt.float32, mybir.dt.float16, mybir.dt.int16
ALU = mybir.AluOpType
ACTF = mybir.ActivationFunctionType

H = WD = 128
C = 64
F = 128
T = 9            # deformable taps
NCORES = 8
ROWS_PER_BAND = 8
BANDS = H // ROWS_PER_BAND          # 16
UNITS = 4                            # 2-row units per band
PXROW = WD                            # 128 px per image row
PAD = 130                            # padded row length for shifted reads
KB = 5                               # K blocks of main conv (576 -> 640)
SLOT = PAD                           # 130 cols per row slot in scm
SCMW = KB * 10 * SLOT                # 6500 cols per band buffer
TOK = 256                            # fp16 elems per 4-corner token (512B)
UIDX = 2 * T * PXROW                 # tokens per 2-row unit = 2304

_CACHE = {}


STAGE = int(os.environ.get("KSTAGE", "4"))


def build_program():
    if "nc" in _CACHE:
        return _CACHE["nc"]
    # The scheduler's cost model prices SWDGE gather descriptor generation at
    # 0.34 ns/desc; measured on HW it is ~7.9 ns/desc. With the default the
    # scheduler thinks gathers are ~20x cheaper than reality and orders
    # gather-waits ahead of independent work, stalling the pipeline. Patch
    # for the duration of trace/compile (schedule hint only; restored after).
    from concourse import hw_specs
    _spec = hw_specs.get_hw_spec("TRN2")
    _orig_ns = _spec.SWDGE_NS_PER_DESCRIPTOR
    _spec.SWDGE_NS_PER_DESCRIPTOR = 7.93
    try:
        nc = _build_program_inner()
    finally:
        _spec.SWDGE_NS_PER_DESCRIPTOR = _orig_ns
    _CACHE["nc"] = nc
    return nc


def _build_program_inner():
    nc = bacc.Bacc("TRN2", target_bir_lowering=False, debug=False)

    # ---- DRAM I/O ----
    z4 = nc.dram_tensor("z4", [H * WD * TOK], F16, kind="ExternalInput").ap()
    xdup = nc.dram_tensor("xdup", [128, PAD * PAD], F16, kind="ExternalInput").ap()
    woffd_in = nc.dram_tensor("woffd", [128, 3 * 18], F16, kind="ExternalInput").ap()
    woffs_in = nc.dram_tensor("woffs", [64, 3 * 18], F16, kind="ExternalInput").ap()
    wm_in = nc.dram_tensor("wm", [128, 45 * 128], F16, kind="ExternalInput").ap()
    cx_in = nc.dram_tensor("cx", [128, H * T], F32, kind="ExternalInput").ap()
    cy_in = nc.dram_tensor("cy", [128, H * T], F32, kind="ExternalInput").ap()
    i32_in = nc.dram_tensor("i128f", [128, 128], F32, kind="ExternalInput").ap()
    i16_in = nc.dram_tensor("i128h", [128, 128], F16, kind="ExternalInput").ap()
    e8_in = nc.dram_tensor("e8", [128, 8 * 128], F32, kind="ExternalInput").ap()
    b_in = nc.dram_tensor("b_main", [128, 1], F32, kind="ExternalInput").ap()
    boff_in = nc.dram_tensor("b_off", [18, 1], F32, kind="ExternalInput").ap()
    out_dram = nc.dram_tensor("out", [H * WD, F], F32, kind="ExternalOutput").ap()
    dbg = nc.dram_tensor("dbg", [128, 4608], F32, kind="ExternalOutput").ap()

    with tile.TileContext(nc) as tc:
        _emit(nc, tc, z4, xdup, woffd_in, woffs_in, wm_in, cx_in, cy_in,
              i32_in, i16_in, e8_in, b_in, boff_in, out_dram, dbg)

    nc.compile()
    return nc


def _emit(nc, tc, z4, xdup_in, woffd_in, woffs_in, wm_in, cx_in, cy_in,
          i32_in, i16_in, e8_in, b_in, boff_in, out_dram, dbg):
    from contextlib import ExitStack
    with ExitStack() as ctx:
        ec = ctx.enter_context
        st = ec(tc.tile_pool(name="static", bufs=1))
        p_offs = ec(tc.tile_pool(name="offs", bufs=3))
        p_offb = ec(tc.tile_pool(name="offb", bufs=2))
        p_math = ec(tc.tile_pool(name="math", bufs=2))
        p_w4 = ec(tc.tile_pool(name="w4c", bufs=2))
        p_idx = ec(tc.tile_pool(name="idx", bufs=2))
        p_gt = ec(tc.tile_pool(name="gt", bufs=3))
        p_P = ec(tc.tile_pool(name="pp", bufs=2))
        p_cmb = ec(tc.tile_pool(name="cmb", bufs=4))
        p_spx = ec(tc.tile_pool(name="spx", bufs=2))
        p_out = ec(tc.tile_pool(name="outp", bufs=2))
        psA = ec(tc.tile_pool(name="psA", bufs=2, space="PSUM"))
        psB = ec(tc.tile_pool(name="psB", bufs=2, space="PSUM"))
        psS = ec(tc.tile_pool(name="psS", bufs=2, space="PSUM"))
        psC = ec(tc.tile_pool(name="psC", bufs=2, space="PSUM"))

        # ---- static loads (HWDGE; keep GpSimd free for gathers) ----
        xdup = st.tile([128, PAD * PAD], F16)
        nc.sync.dma_start(xdup[:], xdup_in)
        woffd = st.tile([128, 54], F16)
        nc.sync.dma_start(woffd[:], woffd_in)
        woffs = st.tile([64, 54], F16)
        nc.sync.dma_start(woffs[:], woffs_in)
        wm = st.tile([128, 45 * 128], F16)
        nc.sync.dma_start(wm[:], wm_in)
        cx = st.tile([128, H * T], F32)
        nc.sync.dma_start(cx[:], cx_in)
        cy = st.tile([128, H * T], F32)
        nc.sync.dma_start(cy[:], cy_in)
        i32 = st.tile([128, 128], F32)
        nc.sync.dma_start(i32[:], i32_in)
        i16t = st.tile([128, 128], F16)
        nc.sync.dma_start(i16t[:], i16_in)
        e8 = st.tile([128, 8 * 128], F32)
        nc.sync.dma_start(e8[:], e8_in)
        bmain = st.tile([128, 1], F32)
        nc.sync.dma_start(bmain[:], b_in)
        boff = st.tile([18, 1], F32)
        nc.sync.dma_start(boff[:], boff_in)

        scm = [st.tile([128, SCMW], F16, tag=f"scm{r}", name=f"scm{r}")
               for r in range(3)]

        tok_src = bass.AP(z4.tensor, 0, [[TOK, H * WD], [1, TOK]])

        def ap_of(tl, off, dims):
            b = tl[:]
            return bass.AP(b.tensor, b.offset + off, [b.ap[0]] + dims)

        def conv_band(b, scm_b):
            """main conv + output transpose for band b reading scm_b."""
            for ch in range(2):          # two 512-px chunks (4 rows each)
                rb = 4 * ch              # starting row within band
                pc = psC.tile([128, 512], F32, tag="conv")
                n_mm = 45
                k = 0
                for s in range(9):
                    sy, sx = s // 3, s % 3
                    for kb in range(KB):
                        kdim = 128 if kb < 4 else 64
                        lhs = wm[0:kdim, (s * KB + kb) * 128:(s * KB + kb + 1) * 128]
                        rhs = ap_of(scm_b, kb * 10 * SLOT + (rb + sy) * SLOT + sx,
                                    [[SLOT, 4], [1, 128]])
                        rhs = bass.AP(rhs.tensor, rhs.offset,
                                      [[rhs.ap[0][0], kdim]] + rhs.ap[1:])
                        nc.tensor.matmul(
                            pc[:].rearrange("f (r x) -> f r x", r=4), lhs, rhs,
                            start=(k == 0), stop=(k == n_mm - 1))
                        k += 1
                outF = p_out.tile([128, 512], F32, tag="outF")
                nc.scalar.activation(outF[:], pc[:], ACTF.Identity,
                                     bias=bmain[:], scale=1.0)
                po = psB.tile([128, 512], F32, tag="b")
                for j in range(4):
                    nc.tensor.transpose(po[:, j * 128:(j + 1) * 128],
                                        outF[:, j * 128:(j + 1) * 128], i32[:])
                osb = p_out.tile([128, 512], F32, tag="osb")
                nc.scalar.activation(osb[:], po[:], ACTF.Copy)
                base = (b * ROWS_PER_BAND + 4 * ch) * PXROW
                dst = bass.AP(out_dram.tensor, base * F,
                              [[F, 128], [PXROW * F, 4], [1, F]])
                nc.sync.dma_start(
                    dst, osb[:].rearrange("p (j f) -> p j f", j=4))

        def front(b):
            """Offsets conv + bilinear weights + gather-index fold for band b.

            Emitted one band ahead of the gather/combine units and boosted in
            scheduler priority so the next band's gather indices are ready
            before the Q7 finishes the current band's gathers.
            Returns (w4cat, idxb) tiles consumed by units(b).
            """
            # ---------- phase A: offsets conv ----------
            offs_cm = []
            for ch in range(2):
                R = b * ROWS_PER_BAND + 4 * ch
                pa = psA.tile([18, 512], F32, tag="a")
                k = 0
                for ky in range(3):
                    rhs_d = ap_of(xdup, (R + ky) * PAD, [[PAD, 4], [1, 128]])
                    nc.tensor.matmul(
                        pa[:].rearrange("m (r x) -> m r x", r=4),
                        woffd[:, ky * 18:(ky + 1) * 18], rhs_d,
                        start=(k == 0), stop=False)
                    k += 1
                    rhs_s = bass.AP(
                        xdup[:].tensor, xdup[:].offset + (R + ky) * PAD + 2,
                        [[xdup[:].ap[0][0], 64], [PAD, 4], [1, 128]])
                    nc.tensor.matmul(
                        pa[:].rearrange("m (r x) -> m r x", r=4),
                        woffs[:, ky * 18:(ky + 1) * 18], rhs_s,
                        start=False, stop=(ky == 2))
                oc = p_offs.tile([18, 512], F32)
                nc.scalar.activation(oc[:], pa[:], ACTF.Identity,
                                     bias=boff[:], scale=1.0)
                offs_cm.append(oc)
            # ---------- offsets transpose to px-major ----------
            pt = psA.tile([128, 144], F32, tag="a")
            for r in range(ROWS_PER_BAND):
                lhs = offs_cm[r // 4][:, (r % 4) * 128:(r % 4 + 1) * 128]
                nc.tensor.matmul(pt[:, r * 18:(r + 1) * 18], lhs, i32[0:18, 0:18],
                                 start=True, stop=True)
            ob = p_offb.tile([128, 144], F32)
            nc.scalar.activation(ob[:], pt[:], ACTF.Copy)

            # ---------- bilinear weights + indices (px-major) ----------
            NW = ROWS_PER_BAND * T  # 72
            offx = ap_of(ob, 0, [[18, 8], [1, 9]])
            offy = ap_of(ob, 9, [[18, 8], [1, 9]])
            cxs = cx[:, b * NW:(b + 1) * NW]
            cys = cy[:, b * NW:(b + 1) * NW]

            def floor_block(off_ap, cs, hi_clip):
                # All single-ALU fp32 ops, no int round-trip: i16->f32 copies
                # and dual MAX,MIN crawl ~10-50x when a Q7 gather is active
                # (SBUF port contention), plain adds/muls/compares do not.
                # clamp(l, 0, hi) without MAX/MIN ops (those crawl too):
                # low via l*(l>0), high via mask-blend, mask mh reused below
                l = p_math.tile([128, NW], F32, tag="l")
                nc.vector.tensor_tensor(l[:], off_ap, cs, ALU.add)
                lw = p_math.tile([128, NW], F32, tag="lw")
                nc.vector.tensor_scalar(lw[:], l[:], 0.0, None, ALU.is_gt)
                nc.vector.tensor_tensor(l[:], l[:], lw[:], ALU.mult)
                mh = p_math.tile([128, NW], F32, tag="mh")
                nc.vector.tensor_scalar(mh[:], l[:], float(hi_clip), None,
                                        ALU.is_lt)
                nc.vector.tensor_tensor(l[:], l[:], mh[:], ALU.mult)
                t2 = p_math.tile([128, NW], F32, tag="t2")
                nc.vector.tensor_scalar(t2[:], mh[:], float(-hi_clip),
                                        float(hi_clip), ALU.mult, ALU.add)
                nc.vector.tensor_tensor(l[:], l[:], t2[:], ALU.add)
                # floor(l) = round_ne(l - 0.5) via the 2^23 magic number;
                # lands in [0, hi-1] so no upper-clip of x0 is needed, and at
                # integral l the (x0=l-1, fx=1) split is bilinear-equivalent
                x0 = p_math.tile([128, NW], F32, tag="x0")
                nc.vector.tensor_scalar(x0[:], l[:], -0.5, None, ALU.add)
                nc.vector.tensor_scalar(x0[:], x0[:], 12582912.0, None, ALU.add)
                nc.vector.tensor_scalar(x0[:], x0[:], -12582912.0, None, ALU.add)
                fx = p_math.tile([128, NW], F32, tag="fx")
                nc.vector.tensor_tensor(fx[:], l[:], x0[:], ALU.subtract)
                # reference zeroes BOTH x-corners when l == hi exactly;
                # mh == (l < hi) is exactly that mask
                fxm = p_math.tile([128, NW], F16, tag="fxm")
                nc.vector.tensor_tensor(fxm[:], fx[:], mh[:], ALU.mult)
                wxa = p_math.tile([128, NW], F16, tag="wxa")
                nc.vector.tensor_scalar(wxa[:], fx[:], -1.0, 1.0, ALU.mult,
                                        ALU.add)
                return x0, fxm, wxa

            x0, fx, wxa = floor_block(offx, cxs, 127)
            y0, fy, wya = floor_block(offy, cys, 127)

            # 4-corner weights interleaved: w4cat[px, 36*r + 4*t + corner]
            # corner order matches z4 token: [y0x0, y1x0, y0x1, y1x1]
            w4cat = p_w4.tile([128, 4 * NW], F16, tag="w4cat")

            def wdst(corner):
                return ap_of(w4cat, corner, [[36, 8], [4, 9]])

            nc.vector.tensor_tensor(wdst(0), wxa[:], wya[:], ALU.mult)
            nc.vector.tensor_tensor(wdst(1), wxa[:], fy[:], ALU.mult)
            nc.vector.tensor_tensor(wdst(2), fx[:], wya[:], ALU.mult)
            nc.vector.tensor_tensor(wdst(3), fx[:], fy[:], ALU.mult)

            i0f = p_w4.tile([128, NW], F32, tag="i0f")
            nc.vector.scalar_tensor_tensor(i0f[:], y0[:], 128.0, x0[:],
                                           op0=ALU.mult, op1=ALU.add)

            # ---------- index fold to wrapped gather layout ----------
            # token i of the band: i = (r*T + t)*128 + px;
            # idx lives at partition px%16, col i//16 = 72*r + 8*t + px//16.
            # One permutation matmul per px-group a: out_a[p, (r,t)] =
            #   i0f[16a + p%16, (r,t)], then strided casts into idxb.
            idxb = p_idx.tile([128, 576], I16)
            for hh in range(2):
                pr = psA.tile([128, 288], F32, tag="a")
                for aa in range(4):
                    a = 4 * hh + aa
                    nc.tensor.matmul(pr[:, aa * 72:(aa + 1) * 72],
                                     e8[:, a * 128:(a + 1) * 128], i0f[:],
                                     start=True, stop=True)
                # dst col = 72r + 8t + a, src col = 72*aa + 9r + t
                dst = ap_of(idxb, 4 * hh, [[72, 8], [8, 9], [1, 4]])
                src = bass.AP(pr[:].tensor, pr[:].offset,
                              [pr[:].ap[0], [9, 8], [1, 9], [72, 4]])
                nc.vector.tensor_copy(dst, src)
            if STAGE <= 1 and b == 0:
                dbgw = p_w4.tile([128, 288], F32, tag="dbgw")
                nc.vector.tensor_copy(dbgw[:], w4cat[:])
                nc.sync.dma_start(dbg[:, 0:288], dbgw[:])
                nc.sync.dma_start(dbg[:, 288:360], i0f[:])
                dbg16 = p_w4.tile([128, 576], F32, tag="dbg16")
                nc.vector.tensor_copy(dbg16[:], idxb[:])
                nc.sync.dma_start(dbg[:, 576:1152], dbg16[:])
            return w4cat, idxb

        def front_hi(b):
            with tc.high_priority(offset=400):
                return front(b)

        fr = front_hi(0)
        for r in range(3):
            nc.vector.memset(scm[r][:], 0)
        for b in range(BANDS):
            scm_b = scm[b % 3]
            w4cat, idxb = fr
            if b + 1 < BANDS:
                fr = front_hi(b + 1)
            if STAGE <= 1:
                continue

            # ---------- per 2-row unit: gather + combine + sampledT ----------
            for u in range(UNITS):
                gt = p_gt.tile([128, 18 * TOK], F16)
                # >1024 idxs per instr needs single_packet=False (HW cap else)
                nc.gpsimd.dma_gather(
                    out_ap=gt[:].rearrange("p (g e) -> p g e", g=18),
                    in_ap=tok_src,
                    idxs_ap=idxb[:, u * 144:(u + 1) * 144],
                    num_idxs=UIDX,
                    num_idxs_reg=UIDX,
                    elem_size=TOK,
                    elem_step=TOK,
                    single_packet=False,
                )
                if STAGE <= 2:
                    if b == 0 and u == 0:
                        dbg16g = p_P.tile([128, 4608], F32, tag="dbgg")
                        nc.vector.tensor_copy(dbg16g[:], gt[:])
                        nc.sync.dma_start(dbg[:, 0:4608], dbg16g[:])
                    continue
                # combine per row-half: P = gt * w4 (broadcast 64) so gt is
                # released after the multiplies, then sum the 4 corners
                P = p_P.tile([128, 4608], F16)
                spx = p_spx.tile([128, 2 * 576], F16)
                for hi in range(2):
                    wsrc = ap_of(w4cat, 72 * u + 36 * hi, [[1, 36], [0, 64]])
                    gh = bass.AP(gt[:].tensor, gt[:].offset + 2304 * hi,
                                 [gt[:].ap[0], [1, 36 * 64]])
                    ph = bass.AP(P[:].tensor, P[:].offset + 2304 * hi,
                                 [P[:].ap[0], [1, 36 * 64]])
                    nc.vector.tensor_tensor(
                        ph.rearrange("p (tc c) -> p tc c", c=64), gh.rearrange(
                            "p (tc c) -> p tc c", c=64), wsrc, ALU.mult)

                    def cs(corner):
                        return ap_of(P, 2304 * hi + corner * 64,
                                     [[256, 9], [1, 64]])

                    t_ab = p_cmb.tile([128, 576], F16, tag="tab")
                    nc.vector.tensor_tensor(
                        t_ab[:].rearrange("p (t c) -> p t c", c=64),
                        cs(0), cs(1), ALU.add)
                    t_cd = p_cmb.tile([128, 576], F16, tag="tcd")
                    nc.vector.tensor_tensor(
                        t_cd[:].rearrange("p (t c) -> p t c", c=64),
                        cs(2), cs(3), ALU.add)
                    nc.vector.tensor_tensor(spx[:, hi * 576:(hi + 1) * 576],
                                            t_ab[:], t_cd[:], ALU.add)

                # sampled transpose to channel-major; boundary rows are also
                # written into the neighbor band's halo slot (replaces
                # explicit halo copies)
                for hi in range(2):
                    slot = 2 * u + hi + 1
                    ps1 = psS.tile([128, 512], F32, tag="s1")
                    ps2 = psB.tile([64, 128], F32, tag="b")
                    for kb in range(4):
                        nc.tensor.matmul(
                            ps1[:, kb * 128:(kb + 1) * 128],
                            spx[:, hi * 576 + kb * 128: hi * 576 + (kb + 1) * 128],
                            i16t[:], start=True, stop=True)
                    nc.tensor.matmul(ps2[:], spx[:, hi * 576 + 512:hi * 576 + 576],
                                     i16t[:], start=True, stop=True)
                    targets = [(scm_b, slot)]
                    if u == 0 and hi == 0 and b > 0:
                        targets.append((scm[(b - 1) % 3], 9))
                    if u == UNITS - 1 and hi == 1 and b + 1 < BANDS:
                        targets.append((scm[(b + 1) % 3], 0))
                    for (scm_t, sl) in targets:
                        dst1 = ap_of(scm_t, sl * SLOT + 1,
                                     [[10 * SLOT, 4], [1, 128]])
                        nc.scalar.activation(dst1, ps1[:].rearrange(
                            "p (k x) -> p k x", k=4), ACTF.Copy)
                        dst2 = bass.AP(scm_t[:].tensor,
                                       scm_t[:].offset + 4 * 10 * SLOT
                                       + sl * SLOT + 1,
                                       [[scm_t[:].ap[0][0], 64], [1, 128]])
                        nc.scalar.activation(dst2, ps2[:], ACTF.Copy)

            if STAGE <= 2:
                continue
            if STAGE <= 3:
                if b == 0:
                    sdbg = p_out.tile([128, 4608], F32, tag="sdbg")
                    nc.vector.tensor_copy(sdbg[:], scm_b[:, 0:4608])
                    nc.sync.dma_start(dbg[:], sdbg[:])
                continue
            # halo slots are filled by the boundary-row double-writes above
            if b > 0:
                conv_band(b - 1, scm[(b - 1) % 3])
            if b == BANDS - 1:
                nc.vector.memset(
                    ap_of(scm_b, 9 * SLOT, [[10 * SLOT, KB], [1, SLOT]]), 0)
                conv_band(b, scm_b)


def _host_prep(x_img, W_off, b_off, W, b):
    """Build per-core input map. x_img: (128,128,64) fp32."""
    C_, T_ = C, T
    # 4-corner duplicated token layout: z4[y, x] =
    #   [x(y,x,:), x(y+1,x,:), x(y,x+1,:), x(y+1,x+1,:)]  (zeros past edges)
    xh = np.ascontiguousarray(x_img, np.float32).astype(np.float16)
    z4 = np.zeros((H, WD, 4, C_), np.float16)
    z4[:, :, 0] = xh
    z4[:H - 1, :, 1] = xh[1:]
    z4[:, :WD - 1, 2] = xh[:, 1:]
    z4[:H - 1, :WD - 1, 3] = xh[1:, 1:]
    z4 = z4.reshape(H * WD * TOK)

    # padded transposed image + dup(+1 col) for offset conv
    xT = np.zeros((C_, PAD, PAD), np.float16)
    xT[:, 1:129, 1:129] = np.transpose(x_img, (2, 0, 1)).astype(np.float16)
    xT = xT.reshape(C_, PAD * PAD)
    xdup = np.zeros((128, PAD * PAD), np.float16)
    xdup[:C_] = xT
    xdup[C_:, :PAD * PAD - 1] = xT[:, 1:]

    perm = list(range(0, 18, 2)) + list(range(1, 18, 2))
    woffd = np.zeros((128, 3 * 18), np.float16)
    woffs = np.zeros((64, 3 * 18), np.float16)
    for ky in range(3):
        woffd[:C_, ky * 18:(ky + 1) * 18] = W_off[ky, 0][:, perm].astype(np.float16)
        woffd[C_:, ky * 18:(ky + 1) * 18] = W_off[ky, 1][:, perm].astype(np.float16)
        woffs[:, ky * 18:(ky + 1) * 18] = W_off[ky, 2][:, perm].astype(np.float16)

    wm = np.zeros((128, 45 * 128), np.float16)
    for s in range(9):
        blk = W[s // 3, s % 3].astype(np.float16)        # [576, 128]
        for kb in range(KB):
            kd = 128 if kb < 4 else 64
            wm[:kd, (s * KB + kb) * 128:(s * KB + kb + 1) * 128] = \
                blk[kb * 128: kb * 128 + kd]

    lo = np.arange(128, dtype=np.float32)
    hi = np.arange(H, dtype=np.float32)
    t = np.arange(T_)
    kx = (t % 3 - 1).astype(np.float32)
    ky = (t // 3 - 1).astype(np.float32)
    cx = (lo[:, None, None] + kx[None, None, :] +
          np.zeros((1, H, 1), np.float32)).reshape(128, H * T_)
    cy = (np.zeros((128, 1, 1), np.float32) + hi[None, :, None] +
          ky[None, None, :]).reshape(128, H * T_)

    e8 = np.zeros((128, 8 * 128), np.float32)
    pp = np.arange(128)
    for a in range(8):
        e8[16 * a + pp % 16, 128 * a + pp] = 1.0

    return dict(
        z4=z4,
        xdup=xdup,
        woffd=woffd,
        woffs=woffs,
        wm=wm,
        cx=np.ascontiguousarray(cx),
        cy=np.ascontiguousarray(cy),
        i128f=np.eye(128, dtype=np.float32),
        i128h=np.eye(128, dtype=np.float16),
        e8=e8,
        b_main=np.asarray(b, np.float32).reshape(128, 1),
        b_off=np.asarray(b_off, np.float32)[
            list(range(0, 18, 2)) + list(range(1, 18, 2))].reshape(18, 1),
    )


def kernel(x, W_off, b_off, W, b, _trace=False):
    x = np.asarray(x, np.float32)
    nc = build_program()
    in_maps = [_host_prep(x[i], np.asarray(W_off, np.float32),
                          np.asarray(b_off, np.float32),
                          np.asarray(W, np.float32),
                          np.asarray(b, np.float32))
               for i in range(NCORES)]
    res = run_bass_kernel_spmd(nc, in_maps, list(range(NCORES)), trace=_trace)
    out = np.stack([res.results[i]["out"].reshape(H, WD, F)
                    for i in range(NCORES)])
    if _trace:
        kernel.last_exec_time_ns = res.exec_time_ns
        kernel.last_results = res
    return out


kernel.last_exec_time_ns = None


# revision 33
# speedup vs baseline: 1.0973x; 1.0642x over previous
"""Deformable Conv2D Trainium2 kernel (8-core data-parallel over batch).

Per core (one image, H=W=128, C=64, F=128, 3x3 deformable conv):
  1. offset conv (PE, fp16, K-packed dual-tap matmuls)
  2. offsets transposed to pixel-major (PE identity matmuls)
  3. bilinear weights (interleaved 4-corner layout) + gather indices (DVE)
  4. index fold to the wrapped gather layout (PE transposes) and
     replication to 128 partitions (PE matmul with a 16->128 rep matrix)
  5. ONE dma_gather per 2-row unit of 512B 4-corner tokens from a
     host-prepped duplicated layout z4[y,x] = [x(y,x), x(y+1,x),
     x(y,x+1), x(y+1,x+1)] fp16 -- 1 token per (pixel, tap)
  6. bilinear combine: 1 broadcast-weight multiply + 3 strided adds per
     unit (DVE, fp16) -> sampled fp16
  7. sampled transposed to channel-major via PE identity matmuls into a
     halo'd per-band buffer
  8. main conv: 45 accumulating PE matmuls per 512-px chunk (fp16)
  9. output transposed to pixel-major (PE transpose-mode), DMA'd out

Self-contained: hardcodes shapes for the nn_DeformableConv2D problem.
"""
import os
import numpy as np

import concourse.bass as bass
import concourse.bacc as bacc
import concourse.tile as tile
from concourse import mybir
from concourse.bass_utils import run_bass_kernel_spmd

F32, F16, I16 = mybir.dt.float32, mybir.dt.float16, mybir.dt.int16
ALU = mybir.AluOpType
ACTF = mybir.ActivationFunctionType

H = WD = 128
C = 64
F = 128
T = 9            # deformable taps
NCORES = 8
ROWS_PER_BAND = 8
BANDS = H // ROWS_PER_BAND          # 16
UNITS = 4                            # 2-row units per band
PXROW = WD                            # 128 px per image row
PAD = 130                            # padded row length for shifted reads
KB = 5                               # K blocks of main conv (576 -> 640)
SLOT = PAD                           # 130 cols per row slot in scm
SCMW = KB * 10 * SLOT                # 6500 cols per band buffer
TOK = 256                            # fp16 elems per 4-corner token (512B)
UIDX = 2 * T * PXROW                 # tokens per 2-row unit = 2304

_CACHE = {}


STAGE = int(os.environ.get("KSTAGE", "4"))


def build_program():
    if "nc" in _CACHE:
        return _CACHE["nc"]
    nc = bacc.Bacc("TRN2", target_bir_lowering=False, debug=False)

    # ---- DRAM I/O ----
    z4 = nc.dram_tensor("z4", [H * WD * TOK], F16, kind="ExternalInput").ap()
    xdup = nc.dram_tensor("xdup", [128, PAD * PAD], F16, kind="ExternalInput").ap()
    woffd_in = nc.dram_tensor("woffd", [128, 3 * 18], F16, kind="ExternalInput").ap()
    woffs_in = nc.dram_tensor("woffs", [64, 3 * 18], F16, kind="ExternalInput").ap()
    wm_in = nc.dram_tensor("wm", [128, 45 * 128], F16, kind="ExternalInput").ap()
    cx_in = nc.dram_tensor("cx", [128, H * T], F32, kind="ExternalInput").ap()
    cy_in = nc.dram_tensor("cy", [128, H * T], F32, kind="ExternalInput").ap()
    i32_in = nc.dram_tensor("i128f", [128, 128], F32, kind="ExternalInput").ap()
    i16_in = nc.dram_tensor("i128h", [128, 128], F16, kind="ExternalInput").ap()
    rep_in = nc.dram_tensor("rep16", [16, 128], F32, kind="ExternalInput").ap()
    b_in = nc.dram_tensor("b_main", [128, 1], F32, kind="ExternalInput").ap()
    boff_in = nc.dram_tensor("b_off", [18, 1], F32, kind="ExternalInput").ap()
    out_dram = nc.dram_tensor("out", [H * WD, F], F32, kind="ExternalOutput").ap()
    dbg = nc.dram_tensor("dbg", [128, 4608], F32, kind="ExternalOutput").ap()

    with tile.TileContext(nc) as tc:
        _emit(nc, tc, z4, xdup, woffd_in, woffs_in, wm_in, cx_in, cy_in,
              i32_in, i16_in, rep_in, b_in, boff_in, out_dram, dbg)

    nc.compile()
    _CACHE["nc"] = nc
    return nc


def _emit(nc, tc, z4, xdup_in, woffd_in, woffs_in, wm_in, cx_in, cy_in,
          i32_in, i16_in, rep_in, b_in, boff_in, out_dram, dbg):
    from contextlib import ExitStack
    with ExitStack() as ctx:
        ec = ctx.enter_context
        st = ec(tc.tile_pool(name="static", bufs=1))
        p_offs = ec(tc.tile_pool(name="offs", bufs=3))
        p_offb = ec(tc.tile_pool(name="offb", bufs=2))
        p_math = ec(tc.tile_pool(name="math", bufs=2))
        p_w4 = ec(tc.tile_pool(name="w4c", bufs=2))
        p_fold = ec(tc.tile_pool(name="fold", bufs=2))
        p_idx = ec(tc.tile_pool(name="idx", bufs=2))
        p_gt = ec(tc.tile_pool(name="gt", bufs=4))
        p_P = ec(tc.tile_pool(name="pp", bufs=2))
        p_cmb = ec(tc.tile_pool(name="cmb", bufs=4))
        p_spx = ec(tc.tile_pool(name="spx", bufs=2))
        p_out = ec(tc.tile_pool(name="outp", bufs=2))
        psA = ec(tc.tile_pool(name="psA", bufs=2, space="PSUM"))
        psB = ec(tc.tile_pool(name="psB", bufs=2, space="PSUM"))
        psS = ec(tc.tile_pool(name="psS", bufs=2, space="PSUM"))
        psC = ec(tc.tile_pool(name="psC", bufs=2, space="PSUM"))

        # ---- static loads (HWDGE; keep GpSimd free for gathers) ----
        xdup = st.tile([128, PAD * PAD], F16)
        nc.sync.dma_start(xdup[:], xdup_in)
        woffd = st.tile([128, 54], F16)
        nc.sync.dma_start(woffd[:], woffd_in)
        woffs = st.tile([64, 54], F16)
        nc.sync.dma_start(woffs[:], woffs_in)
        wm = st.tile([128, 45 * 128], F16)
        nc.sync.dma_start(wm[:], wm_in)
        cx = st.tile([128, H * T], F32)
        nc.sync.dma_start(cx[:], cx_in)
        cy = st.tile([128, H * T], F32)
        nc.sync.dma_start(cy[:], cy_in)
        i32 = st.tile([128, 128], F32)
        nc.sync.dma_start(i32[:], i32_in)
        i16t = st.tile([128, 128], F16)
        nc.sync.dma_start(i16t[:], i16_in)
        rep16 = st.tile([16, 128], F32)
        nc.sync.dma_start(rep16[:], rep_in)
        bmain = st.tile([128, 1], F32)
        nc.sync.dma_start(bmain[:], b_in)
        boff = st.tile([18, 1], F32)
        nc.sync.dma_start(boff[:], boff_in)

        scm = [st.tile([128, SCMW], F16, tag=f"scm{r}", name=f"scm{r}")
               for r in range(3)]

        tok_src = bass.AP(z4.tensor, 0, [[TOK, H * WD], [1, TOK]])

        def ap_of(tl, off, dims):
            b = tl[:]
            return bass.AP(b.tensor, b.offset + off, [b.ap[0]] + dims)

        def conv_band(b, scm_b):
            """main conv + output transpose for band b reading scm_b."""
            for ch in range(2):          # two 512-px chunks (4 rows each)
                rb = 4 * ch              # starting row within band
                pc = psC.tile([128, 512], F32, tag="conv")
                n_mm = 45
                k = 0
                for s in range(9):
                    sy, sx = s // 3, s % 3
                    for kb in range(KB):
                        kdim = 128 if kb < 4 else 64
                        lhs = wm[0:kdim, (s * KB + kb) * 128:(s * KB + kb + 1) * 128]
                        rhs = ap_of(scm_b, kb * 10 * SLOT + (rb + sy) * SLOT + sx,
                                    [[SLOT, 4], [1, 128]])
                        rhs = bass.AP(rhs.tensor, rhs.offset,
                                      [[rhs.ap[0][0], kdim]] + rhs.ap[1:])
                        nc.tensor.matmul(
                            pc[:].rearrange("f (r x) -> f r x", r=4), lhs, rhs,
                            start=(k == 0), stop=(k == n_mm - 1))
                        k += 1
                outF = p_out.tile([128, 512], F32, tag="outF")
                nc.scalar.activation(outF[:], pc[:], ACTF.Identity,
                                     bias=bmain[:], scale=1.0)
                po = psB.tile([128, 512], F32, tag="b")
                for j in range(4):
                    nc.tensor.transpose(po[:, j * 128:(j + 1) * 128],
                                        outF[:, j * 128:(j + 1) * 128], i32[:])
                osb = p_out.tile([128, 512], F32, tag="osb")
                nc.scalar.activation(osb[:], po[:], ACTF.Copy)
                base = (b * ROWS_PER_BAND + 4 * ch) * PXROW
                dst = bass.AP(out_dram.tensor, base * F,
                              [[F, 128], [PXROW * F, 4], [1, F]])
                nc.sync.dma_start(
                    dst, osb[:].rearrange("p (j f) -> p j f", j=4))

        def front(b):
            """Offsets conv + bilinear weights + gather-index fold for band b.

            Emitted one band ahead of the gather/combine units and boosted in
            scheduler priority so the next band's gather indices are ready
            before the Q7 finishes the current band's gathers.
            Returns (w4cat, idxb) tiles consumed by units(b).
            """
            # ---------- phase A: offsets conv ----------
            offs_cm = []
            for ch in range(2):
                R = b * ROWS_PER_BAND + 4 * ch
                pa = psA.tile([18, 512], F32, tag="a")
                k = 0
                for ky in range(3):
                    rhs_d = ap_of(xdup, (R + ky) * PAD, [[PAD, 4], [1, 128]])
                    nc.tensor.matmul(
                        pa[:].rearrange("m (r x) -> m r x", r=4),
                        woffd[:, ky * 18:(ky + 1) * 18], rhs_d,
                        start=(k == 0), stop=False)
                    k += 1
                    rhs_s = bass.AP(
                        xdup[:].tensor, xdup[:].offset + (R + ky) * PAD + 2,
                        [[xdup[:].ap[0][0], 64], [PAD, 4], [1, 128]])
                    nc.tensor.matmul(
                        pa[:].rearrange("m (r x) -> m r x", r=4),
                        woffs[:, ky * 18:(ky + 1) * 18], rhs_s,
                        start=False, stop=(ky == 2))
                oc = p_offs.tile([18, 512], F32)
                nc.scalar.activation(oc[:], pa[:], ACTF.Identity,
                                     bias=boff[:], scale=1.0)
                offs_cm.append(oc)
            # ---------- offsets transpose to px-major ----------
            pt = psA.tile([128, 144], F32, tag="a")
            for r in range(ROWS_PER_BAND):
                lhs = offs_cm[r // 4][:, (r % 4) * 128:(r % 4 + 1) * 128]
                nc.tensor.matmul(pt[:, r * 18:(r + 1) * 18], lhs, i32[0:18, 0:18],
                                 start=True, stop=True)
            ob = p_offb.tile([128, 144], F32)
            nc.scalar.activation(ob[:], pt[:], ACTF.Copy)

            # ---------- bilinear weights + indices (px-major) ----------
            NW = ROWS_PER_BAND * T  # 72
            offx = ap_of(ob, 0, [[18, 8], [1, 9]])
            offy = ap_of(ob, 9, [[18, 8], [1, 9]])
            cxs = cx[:, b * NW:(b + 1) * NW]
            cys = cy[:, b * NW:(b + 1) * NW]

            def floor_block(off_ap, cs, hi_clip):
                l = p_math.tile([128, NW], F32, tag="l")
                nc.vector.tensor_tensor(l[:], off_ap, cs, ALU.add)
                nc.vector.tensor_scalar(l[:], l[:], 0.0, float(hi_clip),
                                        ALU.max, ALU.min)
                xi = p_math.tile([128, NW], I16, tag="xi")
                nc.vector.tensor_copy(xi[:], l[:])
                x0 = p_math.tile([128, NW], F32, tag="x0")
                nc.vector.tensor_copy(x0[:], xi[:])
                cg = p_math.tile([128, NW], F32, tag="cg")
                nc.vector.tensor_tensor(cg[:], x0[:], l[:], ALU.is_gt)
                nc.vector.tensor_tensor(x0[:], x0[:], cg[:], ALU.subtract)
                fx = p_math.tile([128, NW], F32, tag="fx")
                nc.vector.tensor_tensor(fx[:], l[:], x0[:], ALU.subtract)
                mx = p_math.tile([128, NW], F32, tag="mx")
                nc.vector.tensor_scalar(mx[:], x0[:], float(hi_clip - 1), None,
                                        ALU.is_le)
                wxa = p_math.tile([128, NW], F32, tag="wxa")
                nc.vector.tensor_scalar(wxa[:], fx[:], -1.0, 1.0, ALU.mult, ALU.add)
                nc.vector.tensor_tensor(wxa[:], wxa[:], mx[:], ALU.mult)
                return x0, fx, wxa

            x0, fx, wxa = floor_block(offx, cxs, 127)
            y0, fy, wya = floor_block(offy, cys, 127)

            # 4-corner weights interleaved: w4cat[px, 36*r + 4*t + corner]
            # corner order matches z4 token: [y0x0, y1x0, y0x1, y1x1]
            w4cat = p_w4.tile([128, 4 * NW], F16, tag="w4cat")

            def wdst(corner):
                return ap_of(w4cat, corner, [[36, 8], [4, 9]])

            nc.vector.tensor_tensor(wdst(0), wxa[:], wya[:], ALU.mult)
            nc.vector.tensor_tensor(wdst(1), wxa[:], fy[:], ALU.mult)
            nc.vector.tensor_tensor(wdst(2), fx[:], wya[:], ALU.mult)
            nc.vector.tensor_tensor(wdst(3), fx[:], fy[:], ALU.mult)

            i0f = p_w4.tile([128, NW], F32, tag="i0f")
            nc.vector.scalar_tensor_tensor(i0f[:], y0[:], 128.0, x0[:],
                                           op0=ALU.mult, op1=ALU.add)

            # ---------- index fold to wrapped gather layout ----------
            # token i of the band: i = (r*T + t)*128 + px;
            # idx lives at partition px%16, col i//16 = 72*r + 8*t + px//16
            p1 = psA.tile([72, 128], F32, tag="a")
            nc.tensor.matmul(p1[:], i0f[:], i32[:], start=True, stop=True)
            t1 = p_fold.tile([72, 128], F32, tag="t1")
            nc.scalar.activation(t1[:], p1[:], ACTF.Copy)
            idx16f = p_fold.tile([16, 576], F32, tag="idx16f")
            p2a = psA.tile([16, 288], F32, tag="a")
            p2b = psA.tile([16, 288], F32, tag="a")
            for a in range(8):
                p2 = p2a if a < 4 else p2b
                aa = a % 4
                nc.tensor.matmul(p2[:, aa * 72:(aa + 1) * 72],
                                 t1[:, 16 * a:16 * (a + 1)], i32[0:72, 0:72],
                                 start=True, stop=True)
                dst = ap_of(idx16f, a, [[72, 8], [8, 9]])
                nc.scalar.activation(dst, p2[:, aa * 72:(aa + 1) * 72], ACTF.Copy)
            # replicate idxs to all 128 partitions via PE (rep16[k,m]=1 iff m%16==k)
            idxb = p_idx.tile([128, 576], I16)
            for hh in range(2):
                pr = psA.tile([128, 288], F32, tag="a")
                nc.tensor.matmul(pr[:], rep16[:], idx16f[:, hh * 288:(hh + 1) * 288],
                                 start=True, stop=True)
                nc.vector.tensor_copy(idxb[:, hh * 288:(hh + 1) * 288], pr[:])
            if STAGE <= 1 and b == 0:
                dbgw = p_w4.tile([128, 288], F32, tag="dbgw")
                nc.vector.tensor_copy(dbgw[:], w4cat[:])
                nc.sync.dma_start(dbg[:, 0:288], dbgw[:])
                nc.sync.dma_start(dbg[:, 288:360], i0f[:])
                dbg16 = p_w4.tile([128, 576], F32, tag="dbg16")
                nc.vector.tensor_copy(dbg16[:], idxb[:])
                nc.sync.dma_start(dbg[:, 576:1152], dbg16[:])
            return w4cat, idxb

        def front_hi(b):
            with tc.high_priority(offset=400):
                return front(b)

        fr = front_hi(0)
        for r in range(3):
            nc.vector.memset(scm[r][:], 0)
        for b in range(BANDS):
            scm_b = scm[b % 3]
            w4cat, idxb = fr
            if b + 1 < BANDS:
                fr = front_hi(b + 1)
            if STAGE <= 1:
                continue

            # ---------- per 2-row unit: gather + combine + sampledT ----------
            for u in range(UNITS):
                gt = p_gt.tile([128, 18 * TOK], F16)
                # >1024 idxs per instr needs single_packet=False (HW cap else)
                nc.gpsimd.dma_gather(
                    out_ap=gt[:].rearrange("p (g e) -> p g e", g=18),
                    in_ap=tok_src,
                    idxs_ap=idxb[:, u * 144:(u + 1) * 144],
                    num_idxs=UIDX,
                    num_idxs_reg=UIDX,
                    elem_size=TOK,
                    elem_step=TOK,
                    single_packet=False,
                )
                if STAGE <= 2:
                    if b == 0 and u == 0:
                        dbg16g = p_P.tile([128, 4608], F32, tag="dbgg")
                        nc.vector.tensor_copy(dbg16g[:], gt[:])
                        nc.sync.dma_start(dbg[:, 0:4608], dbg16g[:])
                    continue
                # combine: gt *= w4 (broadcast 64, in place), then sum corners
                wsrc = ap_of(w4cat, 72 * u, [[36, 2], [1, 36], [0, 64]])
                nc.vector.tensor_tensor(
                    gt[:].rearrange("p (h tc c) -> p h tc c", h=2, c=64),
                    gt[:].rearrange("p (h tc c) -> p h tc c", h=2, c=64),
                    wsrc, ALU.mult)

                def cslice(corner):
                    return ap_of(gt, corner * 64, [[2304, 2], [256, 9], [1, 64]])

                t_ab = p_cmb.tile([128, 1152], F16, tag="tab")
                nc.vector.tensor_tensor(
                    t_ab[:].rearrange("p (h t c) -> p h t c", h=2, c=64),
                    cslice(0), cslice(1), ALU.add)
                t_cd = p_cmb.tile([128, 1152], F16, tag="tcd")
                nc.vector.tensor_tensor(
                    t_cd[:].rearrange("p (h t c) -> p h t c", h=2, c=64),
                    cslice(2), cslice(3), ALU.add)
                spx = p_spx.tile([128, 2 * 576], F16)
                nc.vector.tensor_tensor(spx[:], t_ab[:], t_cd[:], ALU.add)

                # sampled transpose to channel-major; boundary rows are also
                # written into the neighbor band's halo slot (replaces
                # explicit halo copies)
                for hi in range(2):
                    slot = 2 * u + hi + 1
                    ps1 = psS.tile([128, 512], F32, tag="s1")
                    ps2 = psB.tile([64, 128], F32, tag="b")
                    for kb in range(4):
                        nc.tensor.matmul(
                            ps1[:, kb * 128:(kb + 1) * 128],
                            spx[:, hi * 576 + kb * 128: hi * 576 + (kb + 1) * 128],
                            i16t[:], start=True, stop=True)
                    nc.tensor.matmul(ps2[:], spx[:, hi * 576 + 512:hi * 576 + 576],
                                     i16t[:], start=True, stop=True)
                    targets = [(scm_b, slot)]
                    if u == 0 and hi == 0 and b > 0:
                        targets.append((scm[(b - 1) % 3], 9))
                    if u == UNITS - 1 and hi == 1 and b + 1 < BANDS:
                        targets.append((scm[(b + 1) % 3], 0))
                    for (scm_t, sl) in targets:
                        dst1 = ap_of(scm_t, sl * SLOT + 1,
                                     [[10 * SLOT, 4], [1, 128]])
                        nc.scalar.activation(dst1, ps1[:].rearrange(
                            "p (k x) -> p k x", k=4), ACTF.Copy)
                        dst2 = bass.AP(scm_t[:].tensor,
                                       scm_t[:].offset + 4 * 10 * SLOT
                                       + sl * SLOT + 1,
                                       [[scm_t[:].ap[0][0], 64], [1, 128]])
                        nc.scalar.activation(dst2, ps2[:], ACTF.Copy)

            if STAGE <= 2:
                continue
            if STAGE <= 3:
                if b == 0:
                    sdbg = p_out.tile([128, 4608], F32, tag="sdbg")
                    nc.vector.tensor_copy(sdbg[:], scm_b[:, 0:4608])
                    nc.sync.dma_start(dbg[:], sdbg[:])
                continue
            # halo slots are filled by the boundary-row double-writes above
            if b > 0:
                conv_band(b - 1, scm[(b - 1) % 3])
            if b == BANDS - 1:
                nc.vector.memset(
                    ap_of(scm_b, 9 * SLOT, [[10 * SLOT, KB], [1, SLOT]]), 0)
                conv_band(b, scm_b)


def _host_prep(x_img, W_off, b_off, W, b):
    """Build per-core input map. x_img: (128,128,64) fp32."""
    C_, T_ = C, T
    # 4-corner duplicated token layout: z4[y, x] =
    #   [x(y,x,:), x(y+1,x,:), x(y,x+1,:), x(y+1,x+1,:)]  (zeros past edges)
    xh = np.ascontiguousarray(x_img, np.float32).astype(np.float16)
    z4 = np.zeros((H, WD, 4, C_), np.float16)
    z4[:, :, 0] = xh
    z4[:H - 1, :, 1] = xh[1:]
    z4[:, :WD - 1, 2] = xh[:, 1:]
    z4[:H - 1, :WD - 1, 3] = xh[1:, 1:]
    z4 = z4.reshape(H * WD * TOK)

    # padded transposed image + dup(+1 col) for offset conv
    xT = np.zeros((C_, PAD, PAD), np.float16)
    xT[:, 1:129, 1:129] = np.transpose(x_img, (2, 0, 1)).astype(np.float16)
    xT = xT.reshape(C_, PAD * PAD)
    xdup = np.zeros((128, PAD * PAD), np.float16)
    xdup[:C_] = xT
    xdup[C_:, :PAD * PAD - 1] = xT[:, 1:]

    perm = list(range(0, 18, 2)) + list(range(1, 18, 2))
    woffd = np.zeros((128, 3 * 18), np.float16)
    woffs = np.zeros((64, 3 * 18), np.float16)
    for ky in range(3):
        woffd[:C_, ky * 18:(ky + 1) * 18] = W_off[ky, 0][:, perm].astype(np.float16)
        woffd[C_:, ky * 18:(ky + 1) * 18] = W_off[ky, 1][:, perm].astype(np.float16)
        woffs[:, ky * 18:(ky + 1) * 18] = W_off[ky, 2][:, perm].astype(np.float16)

    wm = np.zeros((128, 45 * 128), np.float16)
    for s in range(9):
        blk = W[s // 3, s % 3].astype(np.float16)        # [576, 128]
        for kb in range(KB):
            kd = 128 if kb < 4 else 64
            wm[:kd, (s * KB + kb) * 128:(s * KB + kb + 1) * 128] = \
                blk[kb * 128: kb * 128 + kd]

    lo = np.arange(128, dtype=np.float32)
    hi = np.arange(H, dtype=np.float32)
    t = np.arange(T_)
    kx = (t % 3 - 1).astype(np.float32)
    ky = (t // 3 - 1).astype(np.float32)
    cx = (lo[:, None, None] + kx[None, None, :] +
          np.zeros((1, H, 1), np.float32)).reshape(128, H * T_)
    cy = (np.zeros((128, 1, 1), np.float32) + hi[None, :, None] +
          ky[None, None, :]).reshape(128, H * T_)

    rep16 = np.zeros((16, 128), np.float32)
    rep16[np.arange(128) % 16, np.arange(128)] = 1.0

    return dict(
        z4=z4,
        xdup=xdup,
        woffd=woffd,
        woffs=woffs,
        wm=wm,
        cx=np.ascontiguousarray(cx),
        cy=np.ascontiguousarray(cy),
        i128f=np.eye(128, dtype=np.float32),
        i128h=np.eye(128, dtype=np.float16),
        rep16=rep16,
        b_main=np.asarray(b, np.float32).reshape(128, 1),
        b_off=np.asarray(b_off, np.float32)[
            list(range(0, 18, 2)) + list(range(1, 18, 2))].reshape(18, 1),
    )


def kernel(x, W_off, b_off, W, b, _trace=False):
    x = np.asarray(x, np.float32)
    nc = build_program()
    in_maps = [_host_prep(x[i], np.asarray(W_off, np.float32),
                          np.asarray(b_off, np.float32),
                          np.asarray(W, np.float32),
                          np.asarray(b, np.float32))
               for i in range(NCORES)]
    res = run_bass_kernel_spmd(nc, in_maps, list(range(NCORES)), trace=_trace)
    out = np.stack([res.results[i]["out"].reshape(H, WD, F)
                    for i in range(NCORES)])
    if _trace:
        kernel.last_exec_time_ns = res.exec_time_ns
        kernel.last_results = res
    return out


kernel.last_exec_time_ns = None


# revision 36
# speedup vs baseline: 1.1319x; 1.0315x over previous
"""Deformable Conv2D Trainium2 kernel (8-core data-parallel over batch).

Per core (one image, H=W=128, C=64, F=128, 3x3 deformable conv):
  1. offset conv (PE, fp16, K-packed dual-tap matmuls)
  2. offsets transposed to pixel-major (PE identity matmuls)
  3. bilinear weights (interleaved 4-corner layout) + gather indices (DVE)
  4. index fold to the wrapped gather layout (PE transposes) and
     replication to 128 partitions (PE matmul with a 16->128 rep matrix)
  5. ONE dma_gather per 2-row unit of 512B 4-corner tokens from a
     host-prepped duplicated layout z4[y,x] = [x(y,x), x(y+1,x),
     x(y,x+1), x(y+1,x+1)] fp16 -- 1 token per (pixel, tap)
  6. bilinear combine: 1 broadcast-weight multiply + 3 strided adds per
     unit (DVE, fp16) -> sampled fp16
  7. sampled transposed to channel-major via PE identity matmuls into a
     halo'd per-band buffer
  8. main conv: 45 accumulating PE matmuls per 512-px chunk (fp16)
  9. output transposed to pixel-major (PE transpose-mode), DMA'd out

Self-contained: hardcodes shapes for the nn_DeformableConv2D problem.
"""
import os
import numpy as np

import concourse.bass as bass
import concourse.bacc as bacc
import concourse.tile as tile
from concourse import mybir
from concourse.bass_utils import run_bass_kernel_spmd

F32, F16, I16 = mybir.dt.float32, mybir.dt.float16, mybir.dt.int16
ALU = mybir.AluOpType
ACTF = mybir.ActivationFunctionType

H = WD = 128
C = 64
F = 128
T = 9            # deformable taps
NCORES = 8
ROWS_PER_BAND = 8
BANDS = H // ROWS_PER_BAND          # 16
UNITS = 4                            # 2-row units per band
PXROW = WD                            # 128 px per image row
PAD = 130                            # padded row length for shifted reads
KB = 5                               # K blocks of main conv (576 -> 640)
SLOT = PAD                           # 130 cols per row slot in scm
SCMW = KB * 10 * SLOT                # 6500 cols per band buffer
TOK = 256                            # fp16 elems per 4-corner token (512B)
UIDX = 2 * T * PXROW                 # tokens per 2-row unit = 2304

_CACHE = {}


STAGE = int(os.environ.get("KSTAGE", "4"))


def build_program():
    if "nc" in _CACHE:
        return _CACHE["nc"]
    nc = bacc.Bacc("TRN2", target_bir_lowering=False, debug=False)

    # ---- DRAM I/O ----
    z4 = nc.dram_tensor("z4", [H * WD * TOK], F16, kind="ExternalInput").ap()
    xdup = nc.dram_tensor("xdup", [128, PAD * PAD], F16, kind="ExternalInput").ap()
    woffd_in = nc.dram_tensor("woffd", [128, 3 * 18], F16, kind="ExternalInput").ap()
    woffs_in = nc.dram_tensor("woffs", [64, 3 * 18], F16, kind="ExternalInput").ap()
    wm_in = nc.dram_tensor("wm", [128, 45 * 128], F16, kind="ExternalInput").ap()
    cx_in = nc.dram_tensor("cx", [128, H * T], F32, kind="ExternalInput").ap()
    cy_in = nc.dram_tensor("cy", [128, H * T], F32, kind="ExternalInput").ap()
    i32_in = nc.dram_tensor("i128f", [128, 128], F32, kind="ExternalInput").ap()
    i16_in = nc.dram_tensor("i128h", [128, 128], F16, kind="ExternalInput").ap()
    rep_in = nc.dram_tensor("rep16", [16, 128], F32, kind="ExternalInput").ap()
    b_in = nc.dram_tensor("b_main", [128, 1], F32, kind="ExternalInput").ap()
    boff_in = nc.dram_tensor("b_off", [18, 1], F32, kind="ExternalInput").ap()
    out_dram = nc.dram_tensor("out", [H * WD, F], F32, kind="ExternalOutput").ap()
    dbg = nc.dram_tensor("dbg", [128, 4608], F32, kind="ExternalOutput").ap()

    with tile.TileContext(nc) as tc:
        _emit(nc, tc, z4, xdup, woffd_in, woffs_in, wm_in, cx_in, cy_in,
              i32_in, i16_in, rep_in, b_in, boff_in, out_dram, dbg)

    nc.compile()
    _CACHE["nc"] = nc
    return nc


def _emit(nc, tc, z4, xdup_in, woffd_in, woffs_in, wm_in, cx_in, cy_in,
          i32_in, i16_in, rep_in, b_in, boff_in, out_dram, dbg):
    from contextlib import ExitStack
    with ExitStack() as ctx:
        ec = ctx.enter_context
        st = ec(tc.tile_pool(name="static", bufs=1))
        p_offs = ec(tc.tile_pool(name="offs", bufs=3))
        p_offb = ec(tc.tile_pool(name="offb", bufs=2))
        p_math = ec(tc.tile_pool(name="math", bufs=2))
        p_w4 = ec(tc.tile_pool(name="w4c", bufs=2))
        p_fold = ec(tc.tile_pool(name="fold", bufs=2))
        p_idx = ec(tc.tile_pool(name="idx", bufs=2))
        p_gt = ec(tc.tile_pool(name="gt", bufs=3))
        p_P = ec(tc.tile_pool(name="pp", bufs=2))
        p_cmb = ec(tc.tile_pool(name="cmb", bufs=4))
        p_spx = ec(tc.tile_pool(name="spx", bufs=2))
        p_out = ec(tc.tile_pool(name="outp", bufs=2))
        psA = ec(tc.tile_pool(name="psA", bufs=2, space="PSUM"))
        psB = ec(tc.tile_pool(name="psB", bufs=2, space="PSUM"))
        psS = ec(tc.tile_pool(name="psS", bufs=2, space="PSUM"))
        psC = ec(tc.tile_pool(name="psC", bufs=2, space="PSUM"))

        # ---- static loads (HWDGE; keep GpSimd free for gathers) ----
        # chunked so band 0's offset conv can start after the first piece
        xdup = st.tile([128, PAD * PAD], F16)
        for cko in range(0, PAD * PAD, 4225):
            ckw = min(4225, PAD * PAD - cko)
            nc.sync.dma_start(xdup[:, cko:cko + ckw],
                              bass.AP(xdup_in.tensor, xdup_in.offset + cko,
                                      [[PAD * PAD, 128], [1, ckw]]))
        woffd = st.tile([128, 54], F16)
        nc.sync.dma_start(woffd[:], woffd_in)
        woffs = st.tile([64, 54], F16)
        nc.sync.dma_start(woffs[:], woffs_in)
        wm = st.tile([128, 45 * 128], F16)
        nc.sync.dma_start(wm[:], wm_in)
        cx = st.tile([128, H * T], F32)
        nc.sync.dma_start(cx[:], cx_in)
        cy = st.tile([128, H * T], F32)
        nc.sync.dma_start(cy[:], cy_in)
        i32 = st.tile([128, 128], F32)
        nc.sync.dma_start(i32[:], i32_in)
        i16t = st.tile([128, 128], F16)
        nc.sync.dma_start(i16t[:], i16_in)
        rep16 = st.tile([16, 128], F32)
        nc.sync.dma_start(rep16[:], rep_in)
        bmain = st.tile([128, 1], F32)
        nc.sync.dma_start(bmain[:], b_in)
        boff = st.tile([18, 1], F32)
        nc.sync.dma_start(boff[:], boff_in)

        scm = [st.tile([128, SCMW], F16, tag=f"scm{r}", name=f"scm{r}")
               for r in range(3)]

        tok_src = bass.AP(z4.tensor, 0, [[TOK, H * WD], [1, TOK]])

        def ap_of(tl, off, dims):
            b = tl[:]
            return bass.AP(b.tensor, b.offset + off, [b.ap[0]] + dims)

        def conv_band(b, scm_b):
            """main conv + output transpose for band b reading scm_b."""
            for ch in range(2):          # two 512-px chunks (4 rows each)
                rb = 4 * ch              # starting row within band
                pc = psC.tile([128, 512], F32, tag="conv")
                n_mm = 45
                k = 0
                for s in range(9):
                    sy, sx = s // 3, s % 3
                    for kb in range(KB):
                        kdim = 128 if kb < 4 else 64
                        lhs = wm[0:kdim, (s * KB + kb) * 128:(s * KB + kb + 1) * 128]
                        rhs = ap_of(scm_b, kb * 10 * SLOT + (rb + sy) * SLOT + sx,
                                    [[SLOT, 4], [1, 128]])
                        rhs = bass.AP(rhs.tensor, rhs.offset,
                                      [[rhs.ap[0][0], kdim]] + rhs.ap[1:])
                        nc.tensor.matmul(
                            pc[:].rearrange("f (r x) -> f r x", r=4), lhs, rhs,
                            start=(k == 0), stop=(k == n_mm - 1))
                        k += 1
                outF = p_out.tile([128, 512], F32, tag="outF")
                nc.scalar.activation(outF[:], pc[:], ACTF.Identity,
                                     bias=bmain[:], scale=1.0)
                po = psB.tile([128, 512], F32, tag="b")
                for j in range(4):
                    nc.tensor.transpose(po[:, j * 128:(j + 1) * 128],
                                        outF[:, j * 128:(j + 1) * 128], i32[:])
                osb = p_out.tile([128, 512], F32, tag="osb")
                nc.scalar.activation(osb[:], po[:], ACTF.Copy)
                base = (b * ROWS_PER_BAND + 4 * ch) * PXROW
                dst = bass.AP(out_dram.tensor, base * F,
                              [[F, 128], [PXROW * F, 4], [1, F]])
                nc.sync.dma_start(
                    dst, osb[:].rearrange("p (j f) -> p j f", j=4))

        def front(b):
            """Offsets conv + bilinear weights + gather-index fold for band b.

            Emitted one band ahead of the gather/combine units and boosted in
            scheduler priority so the next band's gather indices are ready
            before the Q7 finishes the current band's gathers.
            Returns (w4cat, idxb) tiles consumed by units(b).
            """
            # ---------- phase A: offsets conv ----------
            offs_cm = []
            for ch in range(2):
                R = b * ROWS_PER_BAND + 4 * ch
                pa = psA.tile([18, 512], F32, tag="a")
                k = 0
                for ky in range(3):
                    rhs_d = ap_of(xdup, (R + ky) * PAD, [[PAD, 4], [1, 128]])
                    nc.tensor.matmul(
                        pa[:].rearrange("m (r x) -> m r x", r=4),
                        woffd[:, ky * 18:(ky + 1) * 18], rhs_d,
                        start=(k == 0), stop=False)
                    k += 1
                    rhs_s = bass.AP(
                        xdup[:].tensor, xdup[:].offset + (R + ky) * PAD + 2,
                        [[xdup[:].ap[0][0], 64], [PAD, 4], [1, 128]])
                    nc.tensor.matmul(
                        pa[:].rearrange("m (r x) -> m r x", r=4),
                        woffs[:, ky * 18:(ky + 1) * 18], rhs_s,
                        start=False, stop=(ky == 2))
                oc = p_offs.tile([18, 512], F32)
                nc.scalar.activation(oc[:], pa[:], ACTF.Identity,
                                     bias=boff[:], scale=1.0)
                offs_cm.append(oc)
            # ---------- offsets transpose to px-major ----------
            pt = psA.tile([128, 144], F32, tag="a")
            for r in range(ROWS_PER_BAND):
                lhs = offs_cm[r // 4][:, (r % 4) * 128:(r % 4 + 1) * 128]
                nc.tensor.matmul(pt[:, r * 18:(r + 1) * 18], lhs, i32[0:18, 0:18],
                                 start=True, stop=True)
            ob = p_offb.tile([128, 144], F32)
            nc.scalar.activation(ob[:], pt[:], ACTF.Copy)

            # ---------- bilinear weights + indices (px-major) ----------
            NW = ROWS_PER_BAND * T  # 72
            offx = ap_of(ob, 0, [[18, 8], [1, 9]])
            offy = ap_of(ob, 9, [[18, 8], [1, 9]])
            cxs = cx[:, b * NW:(b + 1) * NW]
            cys = cy[:, b * NW:(b + 1) * NW]

            def floor_block(off_ap, cs, hi_clip):
                l = p_math.tile([128, NW], F32, tag="l")
                nc.vector.tensor_tensor(l[:], off_ap, cs, ALU.add)
                nc.vector.tensor_scalar(l[:], l[:], 0.0, float(hi_clip),
                                        ALU.max, ALU.min)
                xi = p_math.tile([128, NW], I16, tag="xi")
                nc.vector.tensor_copy(xi[:], l[:])
                x0 = p_math.tile([128, NW], F32, tag="x0")
                nc.vector.tensor_copy(x0[:], xi[:])
                cg = p_math.tile([128, NW], F32, tag="cg")
                nc.vector.tensor_tensor(cg[:], x0[:], l[:], ALU.is_gt)
                nc.vector.tensor_tensor(x0[:], x0[:], cg[:], ALU.subtract)
                fx = p_math.tile([128, NW], F32, tag="fx")
                nc.vector.tensor_tensor(fx[:], l[:], x0[:], ALU.subtract)
                mx = p_math.tile([128, NW], F32, tag="mx")
                nc.vector.tensor_scalar(mx[:], x0[:], float(hi_clip - 1), None,
                                        ALU.is_le)
                wxa = p_math.tile([128, NW], F32, tag="wxa")
                nc.vector.tensor_scalar(wxa[:], fx[:], -1.0, 1.0, ALU.mult, ALU.add)
                nc.vector.tensor_tensor(wxa[:], wxa[:], mx[:], ALU.mult)
                return x0, fx, wxa

            x0, fx, wxa = floor_block(offx, cxs, 127)
            y0, fy, wya = floor_block(offy, cys, 127)

            # 4-corner weights interleaved: w4cat[px, 36*r + 4*t + corner]
            # corner order matches z4 token: [y0x0, y1x0, y0x1, y1x1]
            w4cat = p_w4.tile([128, 4 * NW], F16, tag="w4cat")

            def wdst(corner):
                return ap_of(w4cat, corner, [[36, 8], [4, 9]])

            nc.vector.tensor_tensor(wdst(0), wxa[:], wya[:], ALU.mult)
            nc.vector.tensor_tensor(wdst(1), wxa[:], fy[:], ALU.mult)
            nc.vector.tensor_tensor(wdst(2), fx[:], wya[:], ALU.mult)
            nc.vector.tensor_tensor(wdst(3), fx[:], fy[:], ALU.mult)

            i0f = p_w4.tile([128, NW], F32, tag="i0f")
            nc.vector.scalar_tensor_tensor(i0f[:], y0[:], 128.0, x0[:],
                                           op0=ALU.mult, op1=ALU.add)

            # ---------- index fold to wrapped gather layout ----------
            # token i of the band: i = (r*T + t)*128 + px;
            # idx lives at partition px%16, col i//16 = 72*r + 8*t + px//16
            p1 = psA.tile([72, 128], F32, tag="a")
            nc.tensor.matmul(p1[:], i0f[:], i32[:], start=True, stop=True)
            t1 = p_fold.tile([72, 128], F32, tag="t1")
            nc.scalar.activation(t1[:], p1[:], ACTF.Copy)
            idx16f = p_fold.tile([16, 576], F32, tag="idx16f")
            p2a = psA.tile([16, 288], F32, tag="a")
            p2b = psA.tile([16, 288], F32, tag="a")
            for a in range(8):
                p2 = p2a if a < 4 else p2b
                aa = a % 4
                nc.tensor.matmul(p2[:, aa * 72:(aa + 1) * 72],
                                 t1[:, 16 * a:16 * (a + 1)], i32[0:72, 0:72],
                                 start=True, stop=True)
                dst = ap_of(idx16f, a, [[72, 8], [8, 9]])
                nc.scalar.activation(dst, p2[:, aa * 72:(aa + 1) * 72], ACTF.Copy)
            # replicate idxs to all 128 partitions via PE (rep16[k,m]=1 iff m%16==k)
            idxb = p_idx.tile([128, 576], I16)
            for hh in range(2):
                pr = psA.tile([128, 288], F32, tag="a")
                nc.tensor.matmul(pr[:], rep16[:], idx16f[:, hh * 288:(hh + 1) * 288],
                                 start=True, stop=True)
                nc.vector.tensor_copy(idxb[:, hh * 288:(hh + 1) * 288], pr[:])
            if STAGE <= 1 and b == 0:
                dbgw = p_w4.tile([128, 288], F32, tag="dbgw")
                nc.vector.tensor_copy(dbgw[:], w4cat[:])
                nc.sync.dma_start(dbg[:, 0:288], dbgw[:])
                nc.sync.dma_start(dbg[:, 288:360], i0f[:])
                dbg16 = p_w4.tile([128, 576], F32, tag="dbg16")
                nc.vector.tensor_copy(dbg16[:], idxb[:])
                nc.sync.dma_start(dbg[:, 576:1152], dbg16[:])
            return w4cat, idxb

        def front_hi(b):
            with tc.high_priority(offset=400):
                return front(b)

        fr = front_hi(0)
        for r in range(3):
            nc.vector.memset(scm[r][:], 0)
        for b in range(BANDS):
            scm_b = scm[b % 3]
            w4cat, idxb = fr
            if b + 1 < BANDS:
                fr = front_hi(b + 1)
            if STAGE <= 1:
                continue

            # ---------- per 2-row unit: gather + combine + sampledT ----------
            for u in range(UNITS):
                gt = p_gt.tile([128, 18 * TOK], F16)
                # >1024 idxs per instr needs single_packet=False (HW cap else)
                nc.gpsimd.dma_gather(
                    out_ap=gt[:].rearrange("p (g e) -> p g e", g=18),
                    in_ap=tok_src,
                    idxs_ap=idxb[:, u * 144:(u + 1) * 144],
                    num_idxs=UIDX,
                    num_idxs_reg=UIDX,
                    elem_size=TOK,
                    elem_step=TOK,
                    single_packet=False,
                )
                if STAGE <= 2:
                    if b == 0 and u == 0:
                        dbg16g = p_P.tile([128, 4608], F32, tag="dbgg")
                        nc.vector.tensor_copy(dbg16g[:], gt[:])
                        nc.sync.dma_start(dbg[:, 0:4608], dbg16g[:])
                    continue
                # combine: P = gt * w4 (broadcast 64) -- separate output so gt
                # is released to the next gather right after the multiply --
                # then sum the 4 corners
                P = p_P.tile([128, 4608], F16)
                wsrc = ap_of(w4cat, 72 * u, [[36, 2], [1, 36], [0, 64]])
                nc.vector.tensor_tensor(
                    P[:].rearrange("p (h tc c) -> p h tc c", h=2, c=64),
                    gt[:].rearrange("p (h tc c) -> p h tc c", h=2, c=64),
                    wsrc, ALU.mult)

                def cslice(corner):
                    return ap_of(P, corner * 64, [[2304, 2], [256, 9], [1, 64]])

                t_ab = p_cmb.tile([128, 1152], F16, tag="tab")
                nc.vector.tensor_tensor(
                    t_ab[:].rearrange("p (h t c) -> p h t c", h=2, c=64),
                    cslice(0), cslice(1), ALU.add)
                t_cd = p_cmb.tile([128, 1152], F16, tag="tcd")
                nc.vector.tensor_tensor(
                    t_cd[:].rearrange("p (h t c) -> p h t c", h=2, c=64),
                    cslice(2), cslice(3), ALU.add)
                spx = p_spx.tile([128, 2 * 576], F16)
                nc.vector.tensor_tensor(spx[:], t_ab[:], t_cd[:], ALU.add)

                # sampled transpose to channel-major; boundary rows are also
                # written into the neighbor band's halo slot (replaces
                # explicit halo copies)
                for hi in range(2):
                    slot = 2 * u + hi + 1
                    ps1 = psS.tile([128, 512], F32, tag="s1")
                    ps2 = psB.tile([64, 128], F32, tag="b")
                    for kb in range(4):
                        nc.tensor.matmul(
                            ps1[:, kb * 128:(kb + 1) * 128],
                            spx[:, hi * 576 + kb * 128: hi * 576 + (kb + 1) * 128],
                            i16t[:], start=True, stop=True)
                    nc.tensor.matmul(ps2[:], spx[:, hi * 576 + 512:hi * 576 + 576],
                                     i16t[:], start=True, stop=True)
                    targets = [(scm_b, slot)]
                    if u == 0 and hi == 0 and b > 0:
                        targets.append((scm[(b - 1) % 3], 9))
                    if u == UNITS - 1 and hi == 1 and b + 1 < BANDS:
                        targets.append((scm[(b + 1) % 3], 0))
                    for (scm_t, sl) in targets:
                        dst1 = ap_of(scm_t, sl * SLOT + 1,
                                     [[10 * SLOT, 4], [1, 128]])
                        nc.scalar.activation(dst1, ps1[:].rearrange(
                            "p (k x) -> p k x", k=4), ACTF.Copy)
                        dst2 = bass.AP(scm_t[:].tensor,
                                       scm_t[:].offset + 4 * 10 * SLOT
                                       + sl * SLOT + 1,
                                       [[scm_t[:].ap[0][0], 64], [1, 128]])
                        nc.scalar.activation(dst2, ps2[:], ACTF.Copy)

            if STAGE <= 2:
                continue
            if STAGE <= 3:
                if b == 0:
                    sdbg = p_out.tile([128, 4608], F32, tag="sdbg")
                    nc.vector.tensor_copy(sdbg[:], scm_b[:, 0:4608])
                    nc.sync.dma_start(dbg[:], sdbg[:])
                continue
            # halo slots are filled by the boundary-row double-writes above
            if b > 0:
                conv_band(b - 1, scm[(b - 1) % 3])
            if b == BANDS - 1:
                nc.vector.memset(
                    ap_of(scm_b, 9 * SLOT, [[10 * SLOT, KB], [1, SLOT]]), 0)
                conv_band(b, scm_b)


def _host_prep(x_img, W_off, b_off, W, b):
    """Build per-core input map. x_img: (128,128,64) fp32."""
    C_, T_ = C, T
    # 4-corner duplicated token layout: z4[y, x] =
    #   [x(y,x,:), x(y+1,x,:), x(y,x+1,:), x(y+1,x+1,:)]  (zeros past edges)
    xh = np.ascontiguousarray(x_img, np.float32).astype(np.float16)
    z4 = np.zeros((H, WD, 4, C_), np.float16)
    z4[:, :, 0] = xh
    z4[:H - 1, :, 1] = xh[1:]
    z4[:, :WD - 1, 2] = xh[:, 1:]
    z4[:H - 1, :WD - 1, 3] = xh[1:, 1:]
    z4 = z4.reshape(H * WD * TOK)

    # padded transposed image + dup(+1 col) for offset conv
    xT = np.zeros((C_, PAD, PAD), np.float16)
    xT[:, 1:129, 1:129] = np.transpose(x_img, (2, 0, 1)).astype(np.float16)
    xT = xT.reshape(C_, PAD * PAD)
    xdup = np.zeros((128, PAD * PAD), np.float16)
    xdup[:C_] = xT
    xdup[C_:, :PAD * PAD - 1] = xT[:, 1:]

    perm = list(range(0, 18, 2)) + list(range(1, 18, 2))
    woffd = np.zeros((128, 3 * 18), np.float16)
    woffs = np.zeros((64, 3 * 18), np.float16)
    for ky in range(3):
        woffd[:C_, ky * 18:(ky + 1) * 18] = W_off[ky, 0][:, perm].astype(np.float16)
        woffd[C_:, ky * 18:(ky + 1) * 18] = W_off[ky, 1][:, perm].astype(np.float16)
        woffs[:, ky * 18:(ky + 1) * 18] = W_off[ky, 2][:, perm].astype(np.float16)

    wm = np.zeros((128, 45 * 128), np.float16)
    for s in range(9):
        blk = W[s // 3, s % 3].astype(np.float16)        # [576, 128]
        for kb in range(KB):
            kd = 128 if kb < 4 else 64
            wm[:kd, (s * KB + kb) * 128:(s * KB + kb + 1) * 128] = \
                blk[kb * 128: kb * 128 + kd]

    lo = np.arange(128, dtype=np.float32)
    hi = np.arange(H, dtype=np.float32)
    t = np.arange(T_)
    kx = (t % 3 - 1).astype(np.float32)
    ky = (t // 3 - 1).astype(np.float32)
    cx = (lo[:, None, None] + kx[None, None, :] +
          np.zeros((1, H, 1), np.float32)).reshape(128, H * T_)
    cy = (np.zeros((128, 1, 1), np.float32) + hi[None, :, None] +
          ky[None, None, :]).reshape(128, H * T_)

    rep16 = np.zeros((16, 128), np.float32)
    rep16[np.arange(128) % 16, np.arange(128)] = 1.0

    return dict(
        z4=z4,
        xdup=xdup,
        woffd=woffd,
        woffs=woffs,
        wm=wm,
        cx=np.ascontiguousarray(cx),
        cy=np.ascontiguousarray(cy),
        i128f=np.eye(128, dtype=np.float32),
        i128h=np.eye(128, dtype=np.float16),
        rep16=rep16,
        b_main=np.asarray(b, np.float32).reshape(128, 1),
        b_off=np.asarray(b_off, np.float32)[
            list(range(0, 18, 2)) + list(range(1, 18, 2))].reshape(18, 1),
    )


def kernel(x, W_off, b_off, W, b, _trace=False):
    x = np.asarray(x, np.float32)
    nc = build_program()
    in_maps = [_host_prep(x[i], np.asarray(W_off, np.float32),
                          np.asarray(b_off, np.float32),
                          np.asarray(W, np.float32),
                          np.asarray(b, np.float32))
               for i in range(NCORES)]
    res = run_bass_kernel_spmd(nc, in_maps, list(range(NCORES)), trace=_trace)
    out = np.stack([res.results[i]["out"].reshape(H, WD, F)
                    for i in range(NCORES)])
    if _trace:
        kernel.last_exec_time_ns = res.exec_time_ns
        kernel.last_results = res
    return out


kernel.last_exec_time_ns = None
